# revision 18
# baseline (speedup 1.0000x reference)
"""Trainium2 Bass kernel for nn_CAB (channel-attention block).

8-way batch-parallel (1 sample per NeuronCore). Per core, fused pipeline:
  conv1x1 (PE, fp16) -> depthwise 3x3 (DVE STT chains, fp16 data / fp32 accum)
  -> PE transposes -> gram S=q@k^T accumulated in PSUM over all 16384 pixels
  -> row/col l2 normalization + per-head softmax (exact, fp32)
  -> fold proj_w through the attention matrix (W_effT) -> out = W_eff @ v.

Math identity used: with attn A (block-diag per head), alpha blending and the
final 1x1 proj conv collapse into one matrix:
  out = proj @ (diag(alpha) A1_bd + diag(1-alpha) A2_bd) @ v = W_eff @ v
so branch-2 work is only needed when alpha != 1 (checked at runtime).
"""

import sys

sys.path.insert(0, "/opt/trn_rl_repo")

import numpy as np
from contextlib import ExitStack

import concourse.bass as bass
import concourse.bacc as bacc
import concourse.tile as tile
import concourse.mybir as mybir
from concourse.bass_utils import run_bass_kernel_spmd

F16 = mybir.dt.float16
F32 = mybir.dt.float32
ALU = mybir.AluOpType
AFT = mybir.ActivationFunctionType

B, C, H, W, HEADS = 8, 192, 128, 128, 8
DH = C // HEADS          # 24
N = H * W                # 16384
MB = 16                  # image rows per megablock
NMB = H // MB            # 8
PADW = W + 2             # 130
SLOTS = MB + 2           # 18 row-slots in padded pre-buffers (halo +-1)
MBF = MB * W             # 2048 free elems per megablock

_CACHE = {}


def _dw_cols(w, order="rc"):
    # (ch,1,3,3) -> (ch,9) fp32, tap t=(dy+1)*3+(dx+1)
    return np.ascontiguousarray(w[:, 0].reshape(w.shape[0], 9).astype(np.float32))


def build_nc(full_path: bool, dbg: bool = False):
    nc = bacc.Bacc("TRN2", target_bir_lowering=False, debug=False, num_devices=8)

    x_d = nc.dram_tensor("x", [C, N], F16, kind="ExternalInput")
    y_d = nc.dram_tensor("y", [C, N], F16, kind="ExternalInput")
    wq_d = nc.dram_tensor("wq", [C, C], F16, kind="ExternalInput")       # [cin, cout]
    wkv_d = nc.dram_tensor("wkv", [C, 2 * C], F16, kind="ExternalInput")  # [cin, cout]
    projr_d = nc.dram_tensor("projr", [C, C], F16, kind="ExternalInput")  # [mid, o]
    dwc_d = nc.dram_tensor("dwc", [1024, 10], F32, kind="ExternalInput")
    miscA_d = nc.dram_tensor("miscA", [96, 8], F32, kind="ExternalInput")
    miscB_d = nc.dram_tensor("miscB", [96, 8], F32, kind="ExternalInput")
    ident_d = nc.dram_tensor("ident", [128, 128], F16, kind="ExternalInput")
    ones_d = nc.dram_tensor("ones96", [1, 96], F32, kind="ExternalInput")
    dmask_d = nc.dram_tensor("dmask", [96, 384], F16, kind="ExternalInput")
    out_d = nc.dram_tensor("out", [C, N], F32, kind="ExternalOutput")
    if dbg:
        dbg_qpre = nc.dram_tensor("dbg_qpre", [128, SLOTS * PADW], F16,
                                  kind="ExternalOutput")
        dbg_qdw = nc.dram_tensor("dbg_qdw", [128, MBF], F16, kind="ExternalOutput")
        dbg_v0 = nc.dram_tensor("dbg_v0", [128, N], F16, kind="ExternalOutput")
        dbg_v1 = nc.dram_tensor("dbg_v1", [64, N], F16, kind="ExternalOutput")
        dbg_S = nc.dram_tensor("dbg_S", [96, 384], F32, kind="ExternalOutput")
        dbg_n = nc.dram_tensor("dbg_n", [128, 3], F32, kind="ExternalOutput")
        dbg_A = nc.dram_tensor("dbg_A", [96, 384], F32, kind="ExternalOutput")
        dbg_We0 = nc.dram_tensor("dbg_We0", [128, 192], F16, kind="ExternalOutput")
        dbg_We1 = nc.dram_tensor("dbg_We1", [64, 192], F16, kind="ExternalOutput")
        dbg_qT = nc.dram_tensor("dbg_qT", [128, 768], F16, kind="ExternalOutput")

    with tile.TileContext(nc) as tc, ExitStack() as ctx:
        const = ctx.enter_context(tc.tile_pool(name="const", bufs=1))
        pers = ctx.enter_context(tc.tile_pool(name="pers", bufs=1))
        xio = ctx.enter_context(tc.tile_pool(name="xio", bufs=(2 if full_path else 3)))
        stg = ctx.enter_context(tc.tile_pool(name="stg", bufs=2))
        convps = ctx.enter_context(
            tc.tile_pool(name="convps", bufs=(2 if full_path else 3), space="PSUM"))
        # PSUM bank budget: convps(2-3) + trps(2) + gramps(2 or 4) <= 8.
        # Phase-2 psum tiles reuse the S1a/S1b tags (sequential lifetimes).
        trps = ctx.enter_context(tc.tile_pool(name="trps", bufs=1, space="PSUM"))
        gramps = ctx.enter_context(tc.tile_pool(name="gramps", bufs=1, space="PSUM"))
        pb_ = 1 if full_path else 2
        dwsc = ctx.enter_context(tc.tile_pool(name="dwsc", bufs=pb_))
        dwout = ctx.enter_context(tc.tile_pool(name="dwout", bufs=pb_))
        tsb = ctx.enter_context(tc.tile_pool(name="tsb", bufs=(1 if full_path else 2)))
        small = ctx.enter_context(tc.tile_pool(name="small", bufs=1))

        # ---------------- constants into SBUF ----------------
        def cload(name, shape, dt, src_ap):
            t = const.tile(shape, dt, tag=name)
            nc.sync.dma_start(t[:], src_ap)
            return t

        wq0 = cload("wq0", [128, C], F16, wq_d[0:128, :])
        wq1 = cload("wq1", [64, C], F16, wq_d[128:192, :])
        wkv0 = cload("wkv0", [128, 2 * C], F16, wkv_d[0:128, :])
        wkv1 = cload("wkv1", [64, 2 * C], F16, wkv_d[128:192, :])
        projrA = cload("projrA", [96, C], F16, projr_d[0:96, :])
        projrB = cload("projrB", [96, C], F16, projr_d[96:192, :])
        ident = cload("ident", [128, 128], F16, ident_d[:, :])
        ones96 = cload("ones96", [1, 96], F32, ones_d[:, :])
        dmask = cload("dmask", [96, 384], F16, dmask_d[:, :])
        miscA = cload("miscA", [96, 8], F32, miscA_d[:, :])
        miscB = cload("miscB", [96, 8], F32, miscB_d[:, :])
        # dw scalar columns: row blocks of 128 in dwc: 0:q0 1:k0 2:v0 3:q1k1
        # 4:v1 5:pos0 6:pos1
        dwq0 = cload("dwq0", [128, 10], F32, dwc_d[0:128, :])
        dwk0 = cload("dwk0", [128, 10], F32, dwc_d[128:256, :])
        dwv0 = cload("dwv0", [128, 10], F32, dwc_d[256:384, :])
        dwq1k1 = cload("dwq1k1", [128, 10], F32, dwc_d[384:512, :])
        dwv1 = cload("dwv1", [64, 10], F32, dwc_d[512:576, :])
        if full_path:
            dwp0 = cload("dwp0", [128, 10], F32, dwc_d[640:768, :])
            dwp1 = cload("dwp1", [64, 10], F32, dwc_d[768:832, :])

        # ---------------- persistent state ----------------
        vres0 = pers.tile([128, N], F16, tag="vres0")
        vres1 = pers.tile([64, N], F16, tag="vres1")

        def prebuf(name, parts):
            bufs = []
            for i in range(2):
                t = pers.tile([parts, SLOTS * PADW], F16, tag=f"{name}{i}")
                # zero the W-pad columns (cols 0 and 129 of each row slot)
                pr = t[:].rearrange("p (r w) -> p r w", w=PADW)
                nc.gpsimd.memset(pr[:, :, 0:1], 0.0)
                nc.gpsimd.memset(pr[:, :, PADW - 1:PADW], 0.0)
                bufs.append(t)
            return bufs

        pb_q0 = prebuf("pbq0", 128)
        pb_k0 = prebuf("pbk0", 128)
        pb_v0 = prebuf("pbv0", 128)
        pb_q1k1 = prebuf("pbq1k1", 128)
        pb_v1 = prebuf("pbv1", 64)
        if full_path:
            pb_p0 = prebuf("pbp0", 128)
            pb_p1 = prebuf("pbp1", 64)

        nrm_q0 = pers.tile([128, NMB], F32, tag="nrmq0")
        nrm_k0 = pers.tile([128, NMB], F32, tag="nrmk0")
        nrm_q1k1 = pers.tile([128, NMB], F32, tag="nrmq1k1")
        if full_path:
            nrm_p0 = pers.tile([128, NMB], F32, tag="nrmp0")
            nrm_p1 = pers.tile([64, NMB], F32, tag="nrmp1")

        S1a = gramps.tile([96, 192], F32, tag="S1a")
        S1b = gramps.tile([96, 192], F32, tag="S1b")
        if full_path:
            S2a = gramps.tile([96, 192], F32, tag="S2a")
            S2b = gramps.tile([96, 192], F32, tag="S2b")

        # =========== PHASE 1: stream over megablocks ===========
        for m in range(NMB):
            r_lo = max(0, MB * m - 1)
            r_hi = min(H, MB * m + MB + 1)
            bi = m % 2
            base = MB * m - 1  # image row of slot 0

            # zero halo slots at image borders
            if m == 0:
                pb_list = [pb_q0, pb_k0, pb_v0, pb_q1k1, pb_v1]
                if full_path:
                    pb_list = pb_list + [pb_p0, pb_p1]
                for pb in pb_list:
                    t = pb[bi]
                    tr = t[:].rearrange("p (r w) -> p r w", w=PADW)
                    nc.gpsimd.memset(tr[:, 0:1, :], 0.0)
            if m == NMB - 1:
                pb_list = [pb_q0, pb_k0, pb_v0, pb_q1k1, pb_v1]
                if full_path:
                    pb_list = pb_list + [pb_p0, pb_p1]
                for pb in pb_list:
                    t = pb[bi]
                    tr = t[:].rearrange("p (r w) -> p r w", w=PADW)
                    nc.gpsimd.memset(tr[:, SLOTS - 1:SLOTS, :], 0.0)

            # ---- conv1x1 into padded pre-buffers, 4-row subtiles ----
            r0 = r_lo
            while r0 < r_hi:
                nr = min(4, r_hi - r0)
                ncols = nr * W
                n0 = r0 * W
                slot0 = r0 - base

                xa = xio.tile([128, 512], F16, tag="xa")
                xb = xio.tile([64, 512], F16, tag="xb")
                ya = xio.tile([128, 512], F16, tag="ya")
                yb = xio.tile([64, 512], F16, tag="yb")
                nc.sync.dma_start(xa[:, 0:ncols], x_d[0:128, n0:n0 + ncols])
                nc.sync.dma_start(xb[:, 0:ncols], x_d[128:192, n0:n0 + ncols])
                nc.sync.dma_start(ya[:, 0:ncols], y_d[0:128, n0:n0 + ncols])
                nc.sync.dma_start(yb[:, 0:ncols], y_d[128:192, n0:n0 + ncols])

                def slot_ap(pb_t, parts, s0, nrr):
                    r = pb_t[0:parts, :].rearrange("p (r w) -> p r w", w=PADW)
                    return r[:, s0:s0 + nrr, 1:1 + W]

                def conv_piece(rhs_a, rhs_b, w0, w1, mo, msz, dst_ap, via_dma=None):
                    ps = convps.tile([128, 512], F32, tag="cps")
                    o = ps[0:msz, 0:ncols]
                    nc.tensor.matmul(o, w0[:, mo:mo + msz], rhs_a[:, 0:ncols],
                                     start=True, stop=False)
                    nc.tensor.matmul(o, w1[:, mo:mo + msz], rhs_b[:, 0:ncols],
                                     start=False, stop=True)
                    if via_dma is None:
                        nc.scalar.copy(dst_ap, o.rearrange("p (r w) -> p r w", w=W))
                    else:
                        s = stg.tile([64, 512], F16, tag="kstg")
                        nc.scalar.copy(s[:, 0:ncols], o)
                        nc.sync.dma_start(
                            dst_ap, s[:, 0:ncols].rearrange("p (r w) -> p r w", w=W))

                # q = Wq @ x
                conv_piece(xa, xb, wq0, wq1, 0, 128,
                           slot_ap(pb_q0[bi], 128, slot0, nr))
                conv_piece(xa, xb, wq0, wq1, 128, 64,
                           slot_ap(pb_q1k1[bi], 64, slot0, nr))
                # k = Wkv[:,0:192] @ y ; v = Wkv[:,192:384] @ y
                conv_piece(ya, yb, wkv0, wkv1, 0, 128,
                           slot_ap(pb_k0[bi], 128, slot0, nr))
                # k1 -> partitions 64:128 of pb_q1k1 via SBUF staging + DMA
                k1_dst = pb_q1k1[bi][64:128, :].rearrange(
                    "p (r w) -> p r w", w=PADW)[:, slot0:slot0 + nr, 1:1 + W]
                conv_piece(ya, yb, wkv0, wkv1, 128, 64, k1_dst, via_dma=True)
                conv_piece(ya, yb, wkv0, wkv1, 192, 128,
                           slot_ap(pb_v0[bi], 128, slot0, nr))
                conv_piece(ya, yb, wkv0, wkv1, 384 - 64, 64,
                           slot_ap(pb_v1[bi], 64, slot0, nr))
                if full_path:
                    nc.scalar.copy(slot_ap(pb_p0[bi], 128, slot0, nr),
                                   xa[:, 0:ncols].rearrange("p (r w) -> p r w", w=W))
                    nc.scalar.copy(slot_ap(pb_p1[bi], 64, slot0, nr),
                                   xb[0:64, 0:ncols].rearrange("p (r w) -> p r w", w=W))
                r0 += nr

            # ---- depthwise 3x3 via STT chains ----
            def dw_chain(pb_t, parts, wcol, dst_ap, bias_col=None):
                pr = pb_t[0:parts, :].rearrange("p (r w) -> p r w", w=PADW)

                def win(t):
                    dy, dx = t // 3 - 1, t % 3 - 1
                    return pr[:, 1 + dy:1 + dy + MB, 1 + dx:1 + dx + W]

                acc_t = dwsc.tile([128, MBF], F32, tag="acc")
                acc = acc_t[0:parts, :].rearrange("p (r w) -> p r w", w=W)
                if bias_col is None:
                    nc.vector.tensor_scalar(acc, win(0), wcol[:, 0:1], None, ALU.mult)
                else:
                    nc.vector.tensor_scalar(acc, win(0), wcol[:, 0:1],
                                            bias_col, ALU.mult, ALU.add)
                for t in range(1, 8):
                    nc.vector.scalar_tensor_tensor(
                        acc, win(t), wcol[:, t:t + 1], acc, ALU.mult, ALU.add)
                nc.vector.scalar_tensor_tensor(
                    dst_ap, win(8), wcol[:, 8:9], acc, ALU.mult, ALU.add)

            if dbg and m == 0:
                nc.sync.dma_start(dbg_qpre[:, :], pb_q0[bi][:])
            qdw = dwout.tile([128, MBF], F16, tag="qdw")
            kdw = dwout.tile([128, MBF], F16, tag="kdw")
            q1k1dw = dwout.tile([128, MBF], F16, tag="q1k1dw")
            r128 = lambda ap: ap.rearrange("p (r w) -> p r w", w=W)
            dw_chain(pb_q0[bi], 128, dwq0, r128(qdw[:]))
            dw_chain(pb_k0[bi], 128, dwk0, r128(kdw[:]))
            dw_chain(pb_q1k1[bi], 128, dwq1k1, r128(q1k1dw[:]))
            dw_chain(pb_v0[bi], 128, dwv0, r128(vres0[:, m * MBF:(m + 1) * MBF]))
            dw_chain(pb_v1[bi], 64, dwv1, r128(vres1[:, m * MBF:(m + 1) * MBF]))
            if full_path:
                pdw = dwout.tile([128, MBF], F16, tag="pdw")
                p1dw = dwout.tile([64, MBF], F16, tag="p1dw")
                dw_chain(pb_p0[bi], 128, dwp0, r128(pdw[:]), bias_col=dwp0[:, 9:10])
                dw_chain(pb_p1[bi], 64, dwp1, r128(p1dw[:]), bias_col=dwp1[:, 9:10])

            if dbg and m == 0:
                nc.sync.dma_start(dbg_qdw[:, :], qdw[:])

            # ---- norms (sum of squares per channel) on ACT ----
            def sq_accum(src_ap, parts, dst_col):
                scr = dwsc.tile([128, MBF], F16, tag="sqscr")
                nc.scalar.activation(scr[0:parts, :], src_ap, AFT.Square,
                                     accum_out=dst_col)
            sq_accum(qdw[:], 128, nrm_q0[:, m:m + 1])
            sq_accum(kdw[:], 128, nrm_k0[:, m:m + 1])
            sq_accum(q1k1dw[:], 128, nrm_q1k1[:, m:m + 1])
            if full_path:
                sq_accum(pdw[:], 128, nrm_p0[:, m:m + 1])
                sq_accum(p1dw[:], 64, nrm_p1[:, m:m + 1])

            # ---- transposes (PE) + gram accumulation ----
            for g in range(4):
                qT_ps = trps.tile([128, 768], F16, tag="qTps")
                kT_ps = trps.tile([128, 768], F16, tag="kTps")
                for r4 in range(4):
                    r = g * 4 + r4
                    sl = slice(r * W, (r + 1) * W)
                    co = r4 * 192
                    nc.tensor.transpose(qT_ps[:, co:co + 128], qdw[:, sl],
                                        ident[:, :])
                    nc.tensor.transpose(qT_ps[:, co + 128:co + 192],
                                        q1k1dw[0:64, sl], ident[0:64, 0:64])
                    nc.tensor.transpose(kT_ps[:, co:co + 128], kdw[:, sl],
                                        ident[:, :])
                    nc.tensor.transpose(kT_ps[:, co + 128:co + 192],
                                        q1k1dw[64:128, sl], ident[64:128, 64:128])
                qT = tsb.tile([128, 768], F16, tag="qT")
                kT = tsb.tile([128, 768], F16, tag="kT")
                nc.scalar.copy(qT[:], qT_ps[:])
                nc.scalar.copy(kT[:], kT_ps[:])
                if full_path:
                    pT_ps = trps.tile([128, 768], F16, tag="qTps")
                    for r4 in range(4):
                        r = g * 4 + r4
                        sl = slice(r * W, (r + 1) * W)
                        co = r4 * 192
                        nc.tensor.transpose(pT_ps[:, co:co + 128], pdw[:, sl],
                                            ident[:, :])
                        nc.tensor.transpose(pT_ps[:, co + 128:co + 192],
                                            p1dw[:, sl], ident[0:64, 0:64])
                    pT = tsb.tile([128, 768], F16, tag="pT")
                    nc.scalar.copy(pT[:], pT_ps[:])
                if dbg and m == 0 and g == 0:
                    nc.sync.dma_start(dbg_qT[:, :], qT[:])
                for r4 in range(4):
                    row = m * MB + g * 4 + r4
                    st = row == 0
                    sp = row == H - 1
                    co = r4 * 192
                    nc.tensor.matmul(S1a[:], qT[:, co:co + 96],
                                     kT[:, co:co + 192], start=st, stop=sp)
                    nc.tensor.matmul(S1b[:], qT[:, co + 96:co + 192],
                                     kT[:, co:co + 192], start=st, stop=sp)
                    if full_path:
                        nc.tensor.matmul(S2a[:], pT[:, co:co + 96],
                                         kT[:, co:co + 192], start=st, stop=sp)
                        nc.tensor.matmul(S2b[:], pT[:, co + 96:co + 192],
                                         kT[:, co:co + 192], start=st, stop=sp)

        # =========== PHASE 2: softmax + W_eff fold (small) ===========
        # Evacuate gram accumulators first so their PSUM tags can be reused.
        Ssb1 = small.tile([96, 384], F32, tag="Ssb1")
        nc.scalar.copy(Ssb1[:, 0:192], S1a[:])
        nc.scalar.copy(Ssb1[:, 192:384], S1b[:])
        if full_path:
            Ssb2 = small.tile([96, 384], F32, tag="Ssb2")
            nc.scalar.copy(Ssb2[:, 0:192], S2a[:])
            nc.scalar.copy(Ssb2[:, 192:384], S2b[:])
        # reduce per-mb sumsq columns -> n^2 per channel
        nq0 = small.tile([128, 1], F32, tag="nq0")
        nk0 = small.tile([128, 1], F32, tag="nk0")
        nq1k1 = small.tile([128, 1], F32, tag="nq1k1")
        nc.vector.tensor_reduce(nq0[:], nrm_q0[:], mybir.AxisListType.X, ALU.add)
        nc.vector.tensor_reduce(nk0[:], nrm_k0[:], mybir.AxisListType.X, ALU.add)
        nc.vector.tensor_reduce(nq1k1[:], nrm_q1k1[:], mybir.AxisListType.X, ALU.add)
        if full_path:
            np0 = small.tile([128, 1], F32, tag="np0")
            np1 = small.tile([64, 1], F32, tag="np1")
            nc.vector.tensor_reduce(np0[:], nrm_p0[:], mybir.AxisListType.X, ALU.add)
            nc.vector.tensor_reduce(np1[:], nrm_p1[:], mybir.AxisListType.X, ALU.add)

        _rs_ctr = [0]

        def rsqrt_col(dst, src_ap, parts):
            # dst = 1 / max(sqrt(src), 1e-12)
            _rs_ctr[0] += 1
            t = small.tile([128, 1], F32, tag=f"rs{_rs_ctr[0]}")
            nc.scalar.sqrt(t[0:parts, :], src_ap)
            nc.vector.tensor_scalar_max(t[0:parts, :], t[0:parts, :], 1e-12)
            nc.vector.reciprocal(dst, t[0:parts, :])
            return dst

        if dbg:
            nc.sync.dma_start(dbg_v0[:, :], vres0[:])
            nc.sync.dma_start(dbg_v1[:, :], vres1[:])
            nc.sync.dma_start(dbg_S[:, :], Ssb1[:])
            nc.sync.dma_start(dbg_n[:, 0:1], nq0[:])
            nc.sync.dma_start(dbg_n[:, 1:2], nk0[:])
            nc.sync.dma_start(dbg_n[:, 2:3], nq1k1[:])
        # q-row scales, head-aligned halves [96,1]
        rqa = small.tile([96, 1], F32, tag="rqa")
        rqb = small.tile([96, 1], F32, tag="rqb")
        nqb = small.tile([96, 1], F32, tag="nqb")
        nc.sync.dma_start(nqb[0:32, :], nq0[96:128, :])
        nc.sync.dma_start(nqb[32:96, :], nq1k1[0:64, :])
        rsqrt_col(rqa[:], nq0[0:96, :], 96)
        rsqrt_col(rqb[:], nqb[:], 96)
        # fold temp1 (per q-channel) into the row scale
        nc.vector.tensor_tensor(rqa[:], rqa[:], miscA[:, 0:1], ALU.mult)
        nc.vector.tensor_tensor(rqb[:], rqb[:], miscB[:, 0:1], ALU.mult)

        # k-col scales as a broadcast tile [96,192]
        nk1 = small.tile([64, 1], F32, tag="nk1")
        nc.sync.dma_start(nk1[:], nq1k1[64:128, :])
        # cast the norm columns to f16 so the PE transpose dtype matches ident
        nk0h = small.tile([128, 1], F16, tag="nk0h")
        nk1h = small.tile([64, 1], F16, tag="nk1h")
        nc.scalar.copy(nk0h[:], nk0[:])
        nc.scalar.copy(nk1h[:], nk1[:])
        rk_ps = gramps.tile([1, 192], F16, tag="S1a")
        nc.tensor.transpose(rk_ps[:, 0:128], nk0h[:], ident[:, :])
        nc.tensor.transpose(rk_ps[:, 128:192], nk1h[:], ident[0:64, 0:64])
        rk_row = small.tile([1, 192], F32, tag="rkrow")
        nc.scalar.sqrt(rk_row[:], rk_ps[:])
        nc.vector.tensor_scalar_max(rk_row[:], rk_row[:], 1e-12)
        nc.vector.reciprocal(rk_row[:], rk_row[:])
        rkb_ps = gramps.tile([96, 192], F32, tag="S1b")
        nc.tensor.matmul(rkb_ps[:], ones96[:], rk_row[:], start=True, stop=True)
        rkb = small.tile([96, 192], F32, tag="rkb")
        nc.scalar.copy(rkb[:], rkb_ps[:])

        def softmax_block(Ssb, rqa_c, rqb_c, tag):
            # Ssb [96,384]: cols 0:192 = q-rows 0:96, 192:384 = q-rows 96:192
            for half, rq_c in ((0, rqa_c), (192, rqb_c)):
                h = Ssb[:, half:half + 192]
                nc.vector.tensor_tensor(h, h, rkb[:], ALU.mult)
                nc.scalar.mul(h, h, rq_c)
            ex = small.tile([96, 384], F32, tag=f"ex_{tag}")
            nc.scalar.activation(ex[:], Ssb[:], AFT.Exp)
            sums = small.tile([96, 16], F32, tag=f"sums_{tag}")
            nc.vector.tensor_reduce(
                sums[:], ex[:].rearrange("p (h j) -> p h j", j=DH),
                mybir.AxisListType.X, ALU.add)
            nc.vector.reciprocal(sums[:], sums[:])
            A = small.tile([96, 384], F32, tag=f"A_{tag}")
            for blk in range(16):
                nc.vector.tensor_scalar_mul(
                    A[:, blk * DH:(blk + 1) * DH], ex[:, blk * DH:(blk + 1) * DH],
                    sums[:, blk:blk + 1])
            return A

        A1 = softmax_block(Ssb1, rqa[:], rqb[:], "a1")

        if dbg:
            nc.sync.dma_start(dbg_A[:, :], A1[:])
        # M_bd [mid, i] block-diagonal, fp16, two partition halves.
        # Build by masking the full softmax tiles (no partition-24 slicing).
        # M1a[mid 0:96, i] = A1a * maskA ; M1b[mid 96:192, i] = A1b * maskB
        M1a = small.tile([96, 192], F16, tag="M1a")
        M1b = small.tile([96, 192], F16, tag="M1b")
        nc.vector.tensor_tensor(M1a[:], A1[:, 0:192], dmask[:, 0:192], ALU.mult)
        nc.vector.tensor_tensor(M1b[:], A1[:, 192:384], dmask[:, 192:384], ALU.mult)

        if full_path:
            # pos-branch scales
            rpa = small.tile([96, 1], F32, tag="rpa")
            rpb = small.tile([96, 1], F32, tag="rpb")
            npb = small.tile([96, 1], F32, tag="npb")
            nc.sync.dma_start(npb[0:32, :], np0[96:128, :])
            nc.sync.dma_start(npb[32:96, :], np1[0:64, :])
            rsqrt_col(rpa[:], np0[0:96, :], 96)
            rsqrt_col(rpb[:], npb[:], 96)
            nc.vector.tensor_tensor(rpa[:], rpa[:], miscA[:, 1:2], ALU.mult)
            nc.vector.tensor_tensor(rpb[:], rpb[:], miscB[:, 1:2], ALU.mult)
            A2 = softmax_block(Ssb2, rpa[:], rpb[:], "a2")
            M2a = small.tile([96, 192], F16, tag="M2a")
            M2b = small.tile([96, 192], F16, tag="M2b")
            nc.vector.tensor_tensor(M2a[:], A2[:, 0:192], dmask[:, 0:192], ALU.mult)
            nc.vector.tensor_tensor(M2b[:], A2[:, 192:384], dmask[:, 192:384], ALU.mult)
            # M = diag(alpha) M1 + diag(1-alpha) M2   (per mid-channel)
            t1 = small.tile([96, 192], F32, tag="mca")
            for Ma, Mb_, mi in ((M1a, M2a, miscA), (M1b, M2b, miscB)):
                nc.vector.tensor_scalar_mul(t1[:], Ma[:], mi[:, 2:3])
                nc.vector.tensor_scalar_mul(Mb_[:], Mb_[:], mi[:, 3:4])
                nc.vector.tensor_tensor(Ma[:], t1[:], Mb_[:], ALU.add)

        # W_effT[i, o] = sum_mid M_bd[mid, i] * projr[mid, o]
        WeT_ps0 = gramps.tile([128, 192], F32, tag="S1a")
        WeT_ps1 = gramps.tile([64, 192], F32, tag="S1b")
        for isl, msz, ps in ((0, 128, WeT_ps0), (128, 64, WeT_ps1)):
            nc.tensor.matmul(ps[:], M1a[:, isl:isl + msz], projrA[:],
                             start=True, stop=False)
            nc.tensor.matmul(ps[:], M1b[:, isl:isl + msz], projrB[:],
                             start=False, stop=True)
        WeT0 = small.tile([128, 192], F16, tag="WeT0")
        WeT1 = small.tile([64, 192], F16, tag="WeT1")
        nc.scalar.copy(WeT0[:], WeT_ps0[:])
        nc.scalar.copy(WeT1[:], WeT_ps1[:])

        if dbg:
            nc.sync.dma_start(dbg_We0[:, :], WeT0[:])
            nc.sync.dma_start(dbg_We1[:, :], WeT1[:])
        # =========== PHASE 3: out = W_eff @ v ===========
        for t in range(N // 512):
            sl = slice(t * 512, (t + 1) * 512)
            big = convps.tile([128, 512], F32, tag="cps")
            sm = convps.tile([64, 512], F32, tag="cps")
            nc.tensor.matmul(big[:], WeT0[:, 0:128], vres0[:, sl],
                             start=True, stop=False)
            nc.tensor.matmul(big[:], WeT1[:, 0:128], vres1[:, sl],
                             start=False, stop=True)
            nc.tensor.matmul(sm[:], WeT0[:, 128:192], vres0[:, sl],
                             start=True, stop=False)
            nc.tensor.matmul(sm[:], WeT1[:, 128:192], vres1[:, sl],
                             start=False, stop=True)
            ob = stg.tile([128, 512], F32, tag="ob")
            os_ = stg.tile([64, 512], F32, tag="os")
            nc.scalar.copy(ob[:], big[:])
            nc.scalar.copy(os_[:], sm[:])
            nc.sync.dma_start(out_d[0:128, sl], ob[:])
            nc.sync.dma_start(out_d[128:192, sl], os_[:])

    nc.compile()
    return nc


def _prep(inputs):
    x = np.asarray(inputs["x"], np.float32)
    y = np.asarray(inputs["y"], np.float32)
    q_w = np.asarray(inputs["q_w"], np.float32)[:, :, 0, 0]      # [out,in]
    kv_w = np.asarray(inputs["kv_w"], np.float32)[:, :, 0, 0]    # [2C,in]
    proj_w = np.asarray(inputs["proj_w"], np.float32)[:, :, 0, 0]
    q_dw = _dw_cols(np.asarray(inputs["q_dw_w"], np.float32))
    kv_dw = _dw_cols(np.asarray(inputs["kv_dw_w"], np.float32))
    pos_dw = _dw_cols(np.asarray(inputs["pos_conv_w"], np.float32))
    temp1 = np.asarray(inputs["temp1"], np.float32).reshape(HEADS)
    temp2 = np.asarray(inputs["temp2"], np.float32).reshape(HEADS)
    alpha = np.asarray(inputs["alpha"], np.float32).reshape(C)
    pos_embed = np.asarray(inputs["pos_embed"], np.float32).reshape(DH)

    full_path = not (np.all(alpha == 1.0))

    k_dw, v_dw = kv_dw[0:C], kv_dw[C:2 * C]
    dwc = np.zeros((1024, 10), np.float32)
    dwc[0:128, 0:9] = q_dw[0:128]
    dwc[128:256, 0:9] = k_dw[0:128]
    dwc[256:384, 0:9] = v_dw[0:128]
    dwc[384:448, 0:9] = q_dw[128:192]
    dwc[448:512, 0:9] = k_dw[128:192]
    dwc[512:576, 0:9] = v_dw[128:192]
    pe_col = np.tile(pos_embed, HEADS)  # per-channel pos_embed
    dwc[640:768, 0:9] = pos_dw[0:128]
    dwc[640:768, 9] = pe_col[0:128]
    dwc[768:832, 0:9] = pos_dw[128:192]
    dwc[768:832, 9] = pe_col[128:192]

    dmask = np.zeros((96, 384), np.float16)
    for h in range(4):
        dmask[h * DH:(h + 1) * DH, h * DH:(h + 1) * DH] = 1.0
    for h in range(4, 8):
        dmask[(h - 4) * DH:(h - 3) * DH, 192 + h * DH:192 + (h + 1) * DH] = 1.0
    tempq = np.repeat(temp1, DH)
    tempp = np.repeat(temp2, DH)
    misc = np.zeros((C, 8), np.float32)
    misc[:, 0] = tempq
    misc[:, 1] = tempp
    misc[:, 2] = alpha
    misc[:, 3] = 1.0 - alpha

    shared = {
        "wq": np.ascontiguousarray(q_w.T.astype(np.float16)),
        "wkv": np.ascontiguousarray(kv_w.T.astype(np.float16)),
        "projr": np.ascontiguousarray(proj_w.T.astype(np.float16)),
        "dwc": dwc,
        "miscA": np.ascontiguousarray(misc[0:96]),
        "miscB": np.ascontiguousarray(misc[96:192]),
        "ident": np.eye(128, dtype=np.float16),
        "ones96": np.ones((1, 96), np.float32),
        "dmask": dmask,
    }
    in_maps = []
    for i in range(B):
        im = dict(shared)
        im["x"] = np.ascontiguousarray(x[i].reshape(C, N).astype(np.float16))
        im["y"] = np.ascontiguousarray(y[i].reshape(C, N).astype(np.float16))
        in_maps.append(im)
    return in_maps, full_path


def kernel(**inputs) -> np.ndarray:
    in_maps, full_path = _prep(inputs)
    if full_path not in _CACHE:
        _CACHE[full_path] = build_nc(full_path)
    nc = _CACHE[full_path]
    res = run_bass_kernel_spmd(nc, in_maps, list(range(B)))
    out = np.stack([res.results[i]["out"].reshape(C, H, W) for i in range(B)])
    return out.astype(np.float32)


if __name__ == "__main__":
    import reference
    inputs = reference.setup_inputs()
    expected = np.asarray(reference.reference(**inputs))
    actual = kernel(**{k: np.asarray(v) for k, v in inputs.items()})
    err = np.abs(actual - expected).max() / (np.abs(expected).max() + 1e-30)
    print("Relative error:", err)


# revision 21
# speedup vs baseline: 1.2477x; 1.2477x over previous
"""Trainium2 Bass kernel for nn_CAB (channel-attention block).

8-way batch-parallel (1 sample per NeuronCore). Per core, fused pipeline:
  conv1x1 (PE, fp16) -> depthwise 3x3 (DVE STT chains, fp16 data / fp32 accum)
  -> PE transposes -> gram S=q@k^T accumulated in PSUM over all 16384 pixels
  -> row/col l2 normalization + per-head softmax (exact, fp32)
  -> fold proj_w through the attention matrix (W_effT) -> out = W_eff @ v.

Math identity used: with attn A (block-diag per head), alpha blending and the
final 1x1 proj conv collapse into one matrix:
  out = proj @ (diag(alpha) A1_bd + diag(1-alpha) A2_bd) @ v = W_eff @ v
so branch-2 work is only needed when alpha != 1 (checked at runtime).
"""

import sys

sys.path.insert(0, "/opt/trn_rl_repo")

import numpy as np
from contextlib import ExitStack

import concourse.bass as bass
import concourse.bacc as bacc
import concourse.tile as tile
import concourse.mybir as mybir
from concourse.bass_utils import run_bass_kernel_spmd

F16 = mybir.dt.float16
F32 = mybir.dt.float32
ALU = mybir.AluOpType
AFT = mybir.ActivationFunctionType

B, C, H, W, HEADS = 8, 192, 128, 128, 8
DH = C // HEADS          # 24
N = H * W                # 16384
MB = 16                  # image rows per megablock
NMB = H // MB            # 8
PADW = W + 2             # 130
SLOTS = MB + 2           # 18 row-slots in padded pre-buffers (halo +-1)
MBF = MB * W             # 2048 free elems per megablock

_CACHE = {}


def _dw_cols(w, order="rc"):
    # (ch,1,3,3) -> (ch,9) fp32, tap t=(dy+1)*3+(dx+1)
    return np.ascontiguousarray(w[:, 0].reshape(w.shape[0], 9).astype(np.float32))


def build_nc(full_path: bool, dbg: bool = False):
    nc = bacc.Bacc("TRN2", target_bir_lowering=False, debug=False, num_devices=8)

    x_d = nc.dram_tensor("x", [C, N], F16, kind="ExternalInput")
    y_d = nc.dram_tensor("y", [C, N], F16, kind="ExternalInput")
    wq_d = nc.dram_tensor("wq", [C, C], F16, kind="ExternalInput")       # [cin, cout]
    wkv_d = nc.dram_tensor("wkv", [C, 2 * C], F16, kind="ExternalInput")  # [cin, cout]
    projr_d = nc.dram_tensor("projr", [C, C], F16, kind="ExternalInput")  # [mid, o]
    dwc_d = nc.dram_tensor("dwc", [1024, 10], F32, kind="ExternalInput")
    miscA_d = nc.dram_tensor("miscA", [96, 8], F32, kind="ExternalInput")
    miscB_d = nc.dram_tensor("miscB", [96, 8], F32, kind="ExternalInput")
    ident_d = nc.dram_tensor("ident", [128, 128], F16, kind="ExternalInput")
    ones_d = nc.dram_tensor("ones96", [1, 96], F32, kind="ExternalInput")
    dmask_d = nc.dram_tensor("dmask", [96, 384], F16, kind="ExternalInput")
    dwdiag_d = nc.dram_tensor("dwdiag", [128, 2304], F16, kind="ExternalInput")
    out_d = nc.dram_tensor("out", [C, N], F32, kind="ExternalOutput")
    if dbg:
        dbg_qpre = nc.dram_tensor("dbg_qpre", [128, SLOTS * PADW], F16,
                                  kind="ExternalOutput")
        dbg_qdw = nc.dram_tensor("dbg_qdw", [128, MBF], F16, kind="ExternalOutput")
        dbg_v0 = nc.dram_tensor("dbg_v0", [128, N], F16, kind="ExternalOutput")
        dbg_v1 = nc.dram_tensor("dbg_v1", [64, N], F16, kind="ExternalOutput")
        dbg_S = nc.dram_tensor("dbg_S", [96, 384], F32, kind="ExternalOutput")
        dbg_n = nc.dram_tensor("dbg_n", [128, 3], F32, kind="ExternalOutput")
        dbg_A = nc.dram_tensor("dbg_A", [96, 384], F32, kind="ExternalOutput")
        dbg_We0 = nc.dram_tensor("dbg_We0", [128, 192], F16, kind="ExternalOutput")
        dbg_We1 = nc.dram_tensor("dbg_We1", [64, 192], F16, kind="ExternalOutput")
        dbg_qT = nc.dram_tensor("dbg_qT", [128, 768], F16, kind="ExternalOutput")

    with tile.TileContext(nc) as tc, ExitStack() as ctx:
        const = ctx.enter_context(tc.tile_pool(name="const", bufs=1))
        pers = ctx.enter_context(tc.tile_pool(name="pers", bufs=1))
        xio = ctx.enter_context(tc.tile_pool(name="xio", bufs=(2 if full_path else 3)))
        stg = ctx.enter_context(tc.tile_pool(name="stg", bufs=2))
        convps = ctx.enter_context(tc.tile_pool(name="convps", bufs=2, space="PSUM"))
        # PSUM bank budget: convps(2-3) + trps(2) + gramps(2 or 4) <= 8.
        # Phase-2 psum tiles reuse the S1a/S1b tags (sequential lifetimes).
        trps = ctx.enter_context(tc.tile_pool(name="trps", bufs=1, space="PSUM"))
        gramps = ctx.enter_context(tc.tile_pool(name="gramps", bufs=1, space="PSUM"))
        pb_ = 1 if full_path else 2
        dwsc = ctx.enter_context(tc.tile_pool(name="dwsc", bufs=pb_))
        dwout = ctx.enter_context(tc.tile_pool(name="dwout", bufs=pb_))
        tsb = ctx.enter_context(tc.tile_pool(name="tsb", bufs=(1 if full_path else 2)))
        small = ctx.enter_context(tc.tile_pool(name="small", bufs=1))

        # ---------------- constants into SBUF ----------------
        def cload(name, shape, dt, src_ap):
            t = const.tile(shape, dt, tag=name)
            nc.sync.dma_start(t[:], src_ap)
            return t

        wq0 = cload("wq0", [128, C], F16, wq_d[0:128, :])
        wq1 = cload("wq1", [64, C], F16, wq_d[128:192, :])
        wkv0 = cload("wkv0", [128, 2 * C], F16, wkv_d[0:128, :])
        wkv1 = cload("wkv1", [64, 2 * C], F16, wkv_d[128:192, :])
        projrA = cload("projrA", [96, C], F16, projr_d[0:96, :])
        projrB = cload("projrB", [96, C], F16, projr_d[96:192, :])
        ident = cload("ident", [128, 128], F16, ident_d[:, :])
        ones96 = cload("ones96", [1, 96], F32, ones_d[:, :])
        dmask = cload("dmask", [96, 384], F16, dmask_d[:, :])
        dwdiag = cload("dwdiag", [128, 2304], F16, dwdiag_d[:, :])
        miscA = cload("miscA", [96, 8], F32, miscA_d[:, :])
        miscB = cload("miscB", [96, 8], F32, miscB_d[:, :])
        # dw scalar columns: row blocks of 128 in dwc: 0:q0 1:k0 2:v0 3:q1k1
        # 4:v1 5:pos0 6:pos1
        dwq0 = cload("dwq0", [128, 10], F32, dwc_d[0:128, :])
        dwk0 = cload("dwk0", [128, 10], F32, dwc_d[128:256, :])
        dwv0 = cload("dwv0", [128, 10], F32, dwc_d[256:384, :])
        dwq1k1 = cload("dwq1k1", [128, 10], F32, dwc_d[384:512, :])
        dwv1 = cload("dwv1", [64, 10], F32, dwc_d[512:576, :])
        if full_path:
            dwp0 = cload("dwp0", [128, 10], F32, dwc_d[640:768, :])
            dwp1 = cload("dwp1", [64, 10], F32, dwc_d[768:832, :])

        # ---------------- persistent state ----------------
        vres0 = pers.tile([128, N], F16, tag="vres0")
        vres1 = pers.tile([64, N], F16, tag="vres1")

        def prebuf(name, parts):
            bufs = []
            for i in range(2):
                t = pers.tile([parts, SLOTS * PADW], F16, tag=f"{name}{i}")
                # zero the W-pad columns (cols 0 and 129 of each row slot)
                pr = t[:].rearrange("p (r w) -> p r w", w=PADW)
                nc.gpsimd.memset(pr[:, :, 0:1], 0.0)
                nc.gpsimd.memset(pr[:, :, PADW - 1:PADW], 0.0)
                bufs.append(t)
            return bufs

        pb_q0 = prebuf("pbq0", 128)
        pb_k0 = prebuf("pbk0", 128)
        pb_v0 = prebuf("pbv0", 128)
        pb_q1k1 = prebuf("pbq1k1", 128)
        pb_v1 = prebuf("pbv1", 64)
        if full_path:
            pb_p0 = prebuf("pbp0", 128)
            pb_p1 = prebuf("pbp1", 64)

        nrm_q0 = pers.tile([128, NMB], F32, tag="nrmq0")
        nrm_k0 = pers.tile([128, NMB], F32, tag="nrmk0")
        nrm_q1k1 = pers.tile([128, NMB], F32, tag="nrmq1k1")
        if full_path:
            nrm_p0 = pers.tile([128, NMB], F32, tag="nrmp0")
            nrm_p1 = pers.tile([64, NMB], F32, tag="nrmp1")

        S1a = gramps.tile([96, 192], F32, tag="S1a")
        S1b = gramps.tile([96, 192], F32, tag="S1b")
        if full_path:
            S2a = gramps.tile([96, 192], F32, tag="S2a")
            S2b = gramps.tile([96, 192], F32, tag="S2b")

        # =========== PHASE 1: stream over megablocks ===========
        for m in range(NMB):
            r_lo = max(0, MB * m - 1)
            r_hi = min(H, MB * m + MB + 1)
            bi = m % 2
            base = MB * m - 1  # image row of slot 0

            # zero halo slots at image borders
            if m == 0:
                pb_list = [pb_q0, pb_k0, pb_v0, pb_q1k1, pb_v1]
                if full_path:
                    pb_list = pb_list + [pb_p0, pb_p1]
                for pb in pb_list:
                    t = pb[bi]
                    tr = t[:].rearrange("p (r w) -> p r w", w=PADW)
                    nc.gpsimd.memset(tr[:, 0:1, :], 0.0)
            if m == NMB - 1:
                pb_list = [pb_q0, pb_k0, pb_v0, pb_q1k1, pb_v1]
                if full_path:
                    pb_list = pb_list + [pb_p0, pb_p1]
                for pb in pb_list:
                    t = pb[bi]
                    tr = t[:].rearrange("p (r w) -> p r w", w=PADW)
                    nc.gpsimd.memset(tr[:, SLOTS - 1:SLOTS, :], 0.0)

            # ---- conv1x1 into padded pre-buffers, 4-row subtiles ----
            r0 = r_lo
            while r0 < r_hi:
                nr = min(4, r_hi - r0)
                ncols = nr * W
                n0 = r0 * W
                slot0 = r0 - base

                xa = xio.tile([128, 512], F16, tag="xa")
                xb = xio.tile([64, 512], F16, tag="xb")
                ya = xio.tile([128, 512], F16, tag="ya")
                yb = xio.tile([64, 512], F16, tag="yb")
                nc.sync.dma_start(xa[:, 0:ncols], x_d[0:128, n0:n0 + ncols])
                nc.sync.dma_start(xb[:, 0:ncols], x_d[128:192, n0:n0 + ncols])
                nc.sync.dma_start(ya[:, 0:ncols], y_d[0:128, n0:n0 + ncols])
                nc.sync.dma_start(yb[:, 0:ncols], y_d[128:192, n0:n0 + ncols])

                def slot_ap(pb_t, parts, s0, nrr):
                    r = pb_t[0:parts, :].rearrange("p (r w) -> p r w", w=PADW)
                    return r[:, s0:s0 + nrr, 1:1 + W]

                def conv_piece(rhs_a, rhs_b, w0, w1, mo, msz, dst_ap, via_dma=None):
                    ps = convps.tile([128, 512], F32, tag="cps")
                    o = ps[0:msz, 0:ncols]
                    nc.tensor.matmul(o, w0[:, mo:mo + msz], rhs_a[:, 0:ncols],
                                     start=True, stop=False)
                    nc.tensor.matmul(o, w1[:, mo:mo + msz], rhs_b[:, 0:ncols],
                                     start=False, stop=True)
                    if via_dma is None:
                        nc.scalar.copy(dst_ap, o.rearrange("p (r w) -> p r w", w=W))
                    else:
                        s = stg.tile([64, 512], F16, tag="kstg")
                        nc.scalar.copy(s[:, 0:ncols], o)
                        nc.sync.dma_start(
                            dst_ap, s[:, 0:ncols].rearrange("p (r w) -> p r w", w=W))

                # q = Wq @ x
                conv_piece(xa, xb, wq0, wq1, 0, 128,
                           slot_ap(pb_q0[bi], 128, slot0, nr))
                conv_piece(xa, xb, wq0, wq1, 128, 64,
                           slot_ap(pb_q1k1[bi], 64, slot0, nr))
                # k = Wkv[:,0:192] @ y ; v = Wkv[:,192:384] @ y
                conv_piece(ya, yb, wkv0, wkv1, 0, 128,
                           slot_ap(pb_k0[bi], 128, slot0, nr))
                # k1 -> partitions 64:128 of pb_q1k1 via SBUF staging + DMA
                k1_dst = pb_q1k1[bi][64:128, :].rearrange(
                    "p (r w) -> p r w", w=PADW)[:, slot0:slot0 + nr, 1:1 + W]
                conv_piece(ya, yb, wkv0, wkv1, 128, 64, k1_dst, via_dma=True)
                conv_piece(ya, yb, wkv0, wkv1, 192, 128,
                           slot_ap(pb_v0[bi], 128, slot0, nr))
                conv_piece(ya, yb, wkv0, wkv1, 384 - 64, 64,
                           slot_ap(pb_v1[bi], 64, slot0, nr))
                if full_path:
                    nc.scalar.copy(slot_ap(pb_p0[bi], 128, slot0, nr),
                                   xa[:, 0:ncols].rearrange("p (r w) -> p r w", w=W))
                    nc.scalar.copy(slot_ap(pb_p1[bi], 64, slot0, nr),
                                   xb[0:64, 0:ncols].rearrange("p (r w) -> p r w", w=W))
                r0 += nr

            # ---- depthwise 3x3: engine-balanced implementations ----
            def dw_win(pb_t, parts, t):
                pr = pb_t[0:parts, :].rearrange("p (r w) -> p r w", w=PADW)
                dy, dx = t // 3 - 1, t % 3 - 1
                return pr[:, 1 + dy:1 + dy + MB, 1 + dx:1 + dx + W]

            def dw_stt(pb_t, parts, wcol, dst_ap, bias_col=None):
                # 9 fused MAC ops on DVE (1x rate), fp32 accumulate
                acc_t = dwsc.tile([128, MBF], F32, tag="acc")
                acc = acc_t[0:parts, :].rearrange("p (r w) -> p r w", w=W)
                if bias_col is None:
                    nc.vector.tensor_scalar(acc, dw_win(pb_t, parts, 0),
                                            wcol[:, 0:1], None, ALU.mult)
                else:
                    nc.vector.tensor_scalar(acc, dw_win(pb_t, parts, 0),
                                            wcol[:, 0:1], bias_col,
                                            ALU.mult, ALU.add)
                for t in range(1, 8):
                    nc.vector.scalar_tensor_tensor(
                        acc, dw_win(pb_t, parts, t), wcol[:, t:t + 1], acc,
                        ALU.mult, ALU.add)
                nc.vector.scalar_tensor_tensor(
                    dst_ap, dw_win(pb_t, parts, 8), wcol[:, 8:9], acc,
                    ALU.mult, ALU.add)

            def dw_split(pb_t, parts, wcol, dst_ap, scale_eng):
                # scaled taps (ACT activation / DVE tensor_scalar 4x)
                # + fp16 add chain on DVE (tensor_tensor 2x)
                sA_t = dwsc.tile([128, MBF], F16, tag="tA")
                sB_t = dwsc.tile([128, MBF], F16, tag="tB")
                sA = sA_t[0:parts, :].rearrange("p (r w) -> p r w", w=W)
                sB = sB_t[0:parts, :].rearrange("p (r w) -> p r w", w=W)

                def scale(dst, t):
                    w_ = dw_win(pb_t, parts, t)
                    if scale_eng == "act":
                        nc.scalar.mul(dst, w_, wcol[:, t:t + 1])
                    else:
                        nc.vector.tensor_scalar(dst, w_, wcol[:, t:t + 1],
                                                None, ALU.mult)

                scale(sA, 0)
                for t in range(1, 8):
                    scale(sB, t)
                    nc.vector.tensor_tensor(sA, sA, sB, ALU.add)
                scale(sB, 8)
                nc.vector.tensor_tensor(dst_ap, sA, sB, ALU.add)

            def dw_pe(pb_t, parts, diag_off, dst_ap_fn):
                # 9 accumulating diag matmuls per 4-row window on PE,
                # ACT evicts PSUM -> fp16 dst
                pr = pb_t[0:parts, :].rearrange("p (r w) -> p r w", w=PADW)
                for gi in range(4):
                    ps = convps.tile([128, 512], F32, tag="dwps")
                    for t in range(9):
                        dy, dx = t // 3 - 1, t % 3 - 1
                        s0 = 1 + 4 * gi + dy
                        rhs = pr[:, s0:s0 + 4, 1 + dx:1 + dx + W]
                        nc.tensor.matmul(
                            ps[0:parts, :],
                            dwdiag[0:parts,
                                   diag_off + t * 128:diag_off + t * 128 + parts],
                            rhs, start=(t == 0), stop=(t == 8))
                    nc.scalar.copy(dst_ap_fn(gi), ps[0:parts, :])

            if dbg and m == 0:
                nc.sync.dma_start(dbg_qpre[:, :], pb_q0[bi][:])
            qdw = dwout.tile([128, MBF], F16, tag="qdw")
            kdw = dwout.tile([128, MBF], F16, tag="kdw")
            q1k1dw = dwout.tile([128, MBF], F16, tag="q1k1dw")
            r128 = lambda ap: ap.rearrange("p (r w) -> p r w", w=W)
            dw_split(pb_q0[bi], 128, dwq0, r128(qdw[:]), "act")
            dw_split(pb_k0[bi], 128, dwk0, r128(kdw[:]), "act")
            dw_pe(pb_q1k1[bi], 128, 1152,
                  lambda gi: q1k1dw[:, gi * 512:(gi + 1) * 512])
            dw_pe(pb_v0[bi], 128, 0,
                  lambda gi: vres0[:, m * MBF + gi * 512:m * MBF + (gi + 1) * 512])
            dw_split(pb_v1[bi], 64, dwv1,
                     r128(vres1[:, m * MBF:(m + 1) * MBF]), "dve")
            if full_path:
                pdw = dwout.tile([128, MBF], F16, tag="pdw")
                p1dw = dwout.tile([64, MBF], F16, tag="p1dw")
                dw_stt(pb_p0[bi], 128, dwp0, r128(pdw[:]), bias_col=dwp0[:, 9:10])
                dw_stt(pb_p1[bi], 64, dwp1, r128(p1dw[:]), bias_col=dwp1[:, 9:10])

            if dbg and m == 0:
                nc.sync.dma_start(dbg_qdw[:, :], qdw[:])

            # ---- norms (sum of squares per channel) on ACT ----
            def sq_accum(src_ap, parts, dst_col):
                scr = dwsc.tile([128, MBF], F16, tag="sqscr")
                nc.scalar.activation(scr[0:parts, :], src_ap, AFT.Square,
                                     accum_out=dst_col)
            sq_accum(qdw[:], 128, nrm_q0[:, m:m + 1])
            sq_accum(kdw[:], 128, nrm_k0[:, m:m + 1])
            sq_accum(q1k1dw[:], 128, nrm_q1k1[:, m:m + 1])
            if full_path:
                sq_accum(pdw[:], 128, nrm_p0[:, m:m + 1])
                sq_accum(p1dw[:], 64, nrm_p1[:, m:m + 1])

            # ---- transposes (PE) + gram accumulation ----
            for g in range(4):
                qT_ps = trps.tile([128, 768], F16, tag="qTps")
                kT_ps = trps.tile([128, 768], F16, tag="kTps")
                for r4 in range(4):
                    r = g * 4 + r4
                    sl = slice(r * W, (r + 1) * W)
                    co = r4 * 192
                    nc.tensor.transpose(qT_ps[:, co:co + 128], qdw[:, sl],
                                        ident[:, :])
                    nc.tensor.transpose(qT_ps[:, co + 128:co + 192],
                                        q1k1dw[0:64, sl], ident[0:64, 0:64])
                    nc.tensor.transpose(kT_ps[:, co:co + 128], kdw[:, sl],
                                        ident[:, :])
                    nc.tensor.transpose(kT_ps[:, co + 128:co + 192],
                                        q1k1dw[64:128, sl], ident[64:128, 64:128])
                qT = tsb.tile([128, 768], F16, tag="qT")
                kT = tsb.tile([128, 768], F16, tag="kT")
                nc.scalar.copy(qT[:], qT_ps[:])
                nc.scalar.copy(kT[:], kT_ps[:])
                if full_path:
                    pT_ps = trps.tile([128, 768], F16, tag="qTps")
                    for r4 in range(4):
                        r = g * 4 + r4
                        sl = slice(r * W, (r + 1) * W)
                        co = r4 * 192
                        nc.tensor.transpose(pT_ps[:, co:co + 128], pdw[:, sl],
                                            ident[:, :])
                        nc.tensor.transpose(pT_ps[:, co + 128:co + 192],
                                            p1dw[:, sl], ident[0:64, 0:64])
                    pT = tsb.tile([128, 768], F16, tag="pT")
                    nc.scalar.copy(pT[:], pT_ps[:])
                if dbg and m == 0 and g == 0:
                    nc.sync.dma_start(dbg_qT[:, :], qT[:])
                for r4 in range(4):
                    row = m * MB + g * 4 + r4
                    st = row == 0
                    sp = row == H - 1
                    co = r4 * 192
                    nc.tensor.matmul(S1a[:], qT[:, co:co + 96],
                                     kT[:, co:co + 192], start=st, stop=sp)
                    nc.tensor.matmul(S1b[:], qT[:, co + 96:co + 192],
                                     kT[:, co:co + 192], start=st, stop=sp)
                    if full_path:
                        nc.tensor.matmul(S2a[:], pT[:, co:co + 96],
                                         kT[:, co:co + 192], start=st, stop=sp)
                        nc.tensor.matmul(S2b[:], pT[:, co + 96:co + 192],
                                         kT[:, co:co + 192], start=st, stop=sp)

        # =========== PHASE 2: softmax + W_eff fold (small) ===========
        # Evacuate gram accumulators first so their PSUM tags can be reused.
        Ssb1 = small.tile([96, 384], F32, tag="Ssb1")
        nc.scalar.copy(Ssb1[:, 0:192], S1a[:])
        nc.scalar.copy(Ssb1[:, 192:384], S1b[:])
        if full_path:
            Ssb2 = small.tile([96, 384], F32, tag="Ssb2")
            nc.scalar.copy(Ssb2[:, 0:192], S2a[:])
            nc.scalar.copy(Ssb2[:, 192:384], S2b[:])
        # reduce per-mb sumsq columns -> n^2 per channel
        nq0 = small.tile([128, 1], F32, tag="nq0")
        nk0 = small.tile([128, 1], F32, tag="nk0")
        nq1k1 = small.tile([128, 1], F32, tag="nq1k1")
        nc.vector.tensor_reduce(nq0[:], nrm_q0[:], mybir.AxisListType.X, ALU.add)
        nc.vector.tensor_reduce(nk0[:], nrm_k0[:], mybir.AxisListType.X, ALU.add)
        nc.vector.tensor_reduce(nq1k1[:], nrm_q1k1[:], mybir.AxisListType.X, ALU.add)
        if full_path:
            np0 = small.tile([128, 1], F32, tag="np0")
            np1 = small.tile([64, 1], F32, tag="np1")
            nc.vector.tensor_reduce(np0[:], nrm_p0[:], mybir.AxisListType.X, ALU.add)
            nc.vector.tensor_reduce(np1[:], nrm_p1[:], mybir.AxisListType.X, ALU.add)

        _rs_ctr = [0]

        def rsqrt_col(dst, src_ap, parts):
            # dst = 1 / max(sqrt(src), 1e-12)
            _rs_ctr[0] += 1
            t = small.tile([128, 1], F32, tag=f"rs{_rs_ctr[0]}")
            nc.scalar.sqrt(t[0:parts, :], src_ap)
            nc.vector.tensor_scalar_max(t[0:parts, :], t[0:parts, :], 1e-12)
            nc.vector.reciprocal(dst, t[0:parts, :])
            return dst

        if dbg:
            nc.sync.dma_start(dbg_v0[:, :], vres0[:])
            nc.sync.dma_start(dbg_v1[:, :], vres1[:])
            nc.sync.dma_start(dbg_S[:, :], Ssb1[:])
            nc.sync.dma_start(dbg_n[:, 0:1], nq0[:])
            nc.sync.dma_start(dbg_n[:, 1:2], nk0[:])
            nc.sync.dma_start(dbg_n[:, 2:3], nq1k1[:])
        # q-row scales, head-aligned halves [96,1]
        rqa = small.tile([96, 1], F32, tag="rqa")
        rqb = small.tile([96, 1], F32, tag="rqb")
        nqb = small.tile([96, 1], F32, tag="nqb")
        nc.sync.dma_start(nqb[0:32, :], nq0[96:128, :])
        nc.sync.dma_start(nqb[32:96, :], nq1k1[0:64, :])
        rsqrt_col(rqa[:], nq0[0:96, :], 96)
        rsqrt_col(rqb[:], nqb[:], 96)
        # fold temp1 (per q-channel) into the row scale
        nc.vector.tensor_tensor(rqa[:], rqa[:], miscA[:, 0:1], ALU.mult)
        nc.vector.tensor_tensor(rqb[:], rqb[:], miscB[:, 0:1], ALU.mult)

        # k-col scales as a broadcast tile [96,192]
        nk1 = small.tile([64, 1], F32, tag="nk1")
        nc.sync.dma_start(nk1[:], nq1k1[64:128, :])
        # cast the norm columns to f16 so the PE transpose dtype matches ident
        nk0h = small.tile([128, 1], F16, tag="nk0h")
        nk1h = small.tile([64, 1], F16, tag="nk1h")
        nc.scalar.copy(nk0h[:], nk0[:])
        nc.scalar.copy(nk1h[:], nk1[:])
        rk_ps = gramps.tile([1, 192], F16, tag="S1a")
        nc.tensor.transpose(rk_ps[:, 0:128], nk0h[:], ident[:, :])
        nc.tensor.transpose(rk_ps[:, 128:192], nk1h[:], ident[0:64, 0:64])
        rk_row = small.tile([1, 192], F32, tag="rkrow")
        nc.scalar.sqrt(rk_row[:], rk_ps[:])
        nc.vector.tensor_scalar_max(rk_row[:], rk_row[:], 1e-12)
        nc.vector.reciprocal(rk_row[:], rk_row[:])
        rkb_ps = gramps.tile([96, 192], F32, tag="S1b")
        nc.tensor.matmul(rkb_ps[:], ones96[:], rk_row[:], start=True, stop=True)
        rkb = small.tile([96, 192], F32, tag="rkb")
        nc.scalar.copy(rkb[:], rkb_ps[:])

        def softmax_block(Ssb, rqa_c, rqb_c, tag):
            # Ssb [96,384]: cols 0:192 = q-rows 0:96, 192:384 = q-rows 96:192
            for half, rq_c in ((0, rqa_c), (192, rqb_c)):
                h = Ssb[:, half:half + 192]
                nc.vector.tensor_tensor(h, h, rkb[:], ALU.mult)
                nc.scalar.mul(h, h, rq_c)
            ex = small.tile([96, 384], F32, tag=f"ex_{tag}")
            nc.scalar.activation(ex[:], Ssb[:], AFT.Exp)
            sums = small.tile([96, 16], F32, tag=f"sums_{tag}")
            nc.vector.tensor_reduce(
                sums[:], ex[:].rearrange("p (h j) -> p h j", j=DH),
                mybir.AxisListType.X, ALU.add)
            nc.vector.reciprocal(sums[:], sums[:])
            A = small.tile([96, 384], F32, tag=f"A_{tag}")
            for blk in range(16):
                nc.vector.tensor_scalar_mul(
                    A[:, blk * DH:(blk + 1) * DH], ex[:, blk * DH:(blk + 1) * DH],
                    sums[:, blk:blk + 1])
            return A

        A1 = softmax_block(Ssb1, rqa[:], rqb[:], "a1")

        if dbg:
            nc.sync.dma_start(dbg_A[:, :], A1[:])
        # M_bd [mid, i] block-diagonal, fp16, two partition halves.
        # Build by masking the full softmax tiles (no partition-24 slicing).
        # M1a[mid 0:96, i] = A1a * maskA ; M1b[mid 96:192, i] = A1b * maskB
        M1a = small.tile([96, 192], F16, tag="M1a")
        M1b = small.tile([96, 192], F16, tag="M1b")
        nc.vector.tensor_tensor(M1a[:], A1[:, 0:192], dmask[:, 0:192], ALU.mult)
        nc.vector.tensor_tensor(M1b[:], A1[:, 192:384], dmask[:, 192:384], ALU.mult)

        if full_path:
            # pos-branch scales
            rpa = small.tile([96, 1], F32, tag="rpa")
            rpb = small.tile([96, 1], F32, tag="rpb")
            npb = small.tile([96, 1], F32, tag="npb")
            nc.sync.dma_start(npb[0:32, :], np0[96:128, :])
            nc.sync.dma_start(npb[32:96, :], np1[0:64, :])
            rsqrt_col(rpa[:], np0[0:96, :], 96)
            rsqrt_col(rpb[:], npb[:], 96)
            nc.vector.tensor_tensor(rpa[:], rpa[:], miscA[:, 1:2], ALU.mult)
            nc.vector.tensor_tensor(rpb[:], rpb[:], miscB[:, 1:2], ALU.mult)
            A2 = softmax_block(Ssb2, rpa[:], rpb[:], "a2")
            M2a = small.tile([96, 192], F16, tag="M2a")
            M2b = small.tile([96, 192], F16, tag="M2b")
            nc.vector.tensor_tensor(M2a[:], A2[:, 0:192], dmask[:, 0:192], ALU.mult)
            nc.vector.tensor_tensor(M2b[:], A2[:, 192:384], dmask[:, 192:384], ALU.mult)
            # M = diag(alpha) M1 + diag(1-alpha) M2   (per mid-channel)
            t1 = small.tile([96, 192], F32, tag="mca")
            for Ma, Mb_, mi in ((M1a, M2a, miscA), (M1b, M2b, miscB)):
                nc.vector.tensor_scalar_mul(t1[:], Ma[:], mi[:, 2:3])
                nc.vector.tensor_scalar_mul(Mb_[:], Mb_[:], mi[:, 3:4])
                nc.vector.tensor_tensor(Ma[:], t1[:], Mb_[:], ALU.add)

        # W_effT[i, o] = sum_mid M_bd[mid, i] * projr[mid, o]
        WeT_ps0 = gramps.tile([128, 192], F32, tag="S1a")
        WeT_ps1 = gramps.tile([64, 192], F32, tag="S1b")
        for isl, msz, ps in ((0, 128, WeT_ps0), (128, 64, WeT_ps1)):
            nc.tensor.matmul(ps[:], M1a[:, isl:isl + msz], projrA[:],
                             start=True, stop=False)
            nc.tensor.matmul(ps[:], M1b[:, isl:isl + msz], projrB[:],
                             start=False, stop=True)
        WeT0 = small.tile([128, 192], F16, tag="WeT0")
        WeT1 = small.tile([64, 192], F16, tag="WeT1")
        nc.scalar.copy(WeT0[:], WeT_ps0[:])
        nc.scalar.copy(WeT1[:], WeT_ps1[:])

        if dbg:
            nc.sync.dma_start(dbg_We0[:, :], WeT0[:])
            nc.sync.dma_start(dbg_We1[:, :], WeT1[:])
        # =========== PHASE 3: out = W_eff @ v ===========
        for t in range(N // 512):
            sl = slice(t * 512, (t + 1) * 512)
            big = convps.tile([128, 512], F32, tag="cps")
            sm = convps.tile([64, 512], F32, tag="cps")
            nc.tensor.matmul(big[:], WeT0[:, 0:128], vres0[:, sl],
                             start=True, stop=False)
            nc.tensor.matmul(big[:], WeT1[:, 0:128], vres1[:, sl],
                             start=False, stop=True)
            nc.tensor.matmul(sm[:], WeT0[:, 128:192], vres0[:, sl],
                             start=True, stop=False)
            nc.tensor.matmul(sm[:], WeT1[:, 128:192], vres1[:, sl],
                             start=False, stop=True)
            ob = stg.tile([128, 512], F32, tag="ob")
            os_ = stg.tile([64, 512], F32, tag="os")
            nc.scalar.copy(ob[:], big[:])
            nc.vector.tensor_copy(os_[:], sm[:])
            nc.sync.dma_start(out_d[0:128, sl], ob[:])
            nc.sync.dma_start(out_d[128:192, sl], os_[:])

    nc.compile()
    return nc


def _prep(inputs):
    x = np.asarray(inputs["x"], np.float32)
    y = np.asarray(inputs["y"], np.float32)
    q_w = np.asarray(inputs["q_w"], np.float32)[:, :, 0, 0]      # [out,in]
    kv_w = np.asarray(inputs["kv_w"], np.float32)[:, :, 0, 0]    # [2C,in]
    proj_w = np.asarray(inputs["proj_w"], np.float32)[:, :, 0, 0]
    q_dw = _dw_cols(np.asarray(inputs["q_dw_w"], np.float32))
    kv_dw = _dw_cols(np.asarray(inputs["kv_dw_w"], np.float32))
    pos_dw = _dw_cols(np.asarray(inputs["pos_conv_w"], np.float32))
    temp1 = np.asarray(inputs["temp1"], np.float32).reshape(HEADS)
    temp2 = np.asarray(inputs["temp2"], np.float32).reshape(HEADS)
    alpha = np.asarray(inputs["alpha"], np.float32).reshape(C)
    pos_embed = np.asarray(inputs["pos_embed"], np.float32).reshape(DH)

    full_path = not (np.all(alpha == 1.0))

    k_dw, v_dw = kv_dw[0:C], kv_dw[C:2 * C]
    dwc = np.zeros((1024, 10), np.float32)
    dwc[0:128, 0:9] = q_dw[0:128]
    dwc[128:256, 0:9] = k_dw[0:128]
    dwc[256:384, 0:9] = v_dw[0:128]
    dwc[384:448, 0:9] = q_dw[128:192]
    dwc[448:512, 0:9] = k_dw[128:192]
    dwc[512:576, 0:9] = v_dw[128:192]
    pe_col = np.tile(pos_embed, HEADS)  # per-channel pos_embed
    dwc[640:768, 0:9] = pos_dw[0:128]
    dwc[640:768, 9] = pe_col[0:128]
    dwc[768:832, 0:9] = pos_dw[128:192]
    dwc[768:832, 9] = pe_col[128:192]

    dmask = np.zeros((96, 384), np.float16)
    for h in range(4):
        dmask[h * DH:(h + 1) * DH, h * DH:(h + 1) * DH] = 1.0
    for h in range(4, 8):
        dmask[(h - 4) * DH:(h - 3) * DH, 192 + h * DH:192 + (h + 1) * DH] = 1.0
    dwdiag = np.zeros((128, 2304), np.float16)
    q1k1_w = np.concatenate([q_dw[128:192], k_dw[128:192]], 0)  # [128, 9]
    for t in range(9):
        np.fill_diagonal(dwdiag[:, t * 128:(t + 1) * 128], v_dw[0:128, t])
        np.fill_diagonal(dwdiag[:, 1152 + t * 128:1152 + (t + 1) * 128],
                         q1k1_w[:, t])
    tempq = np.repeat(temp1, DH)
    tempp = np.repeat(temp2, DH)
    misc = np.zeros((C, 8), np.float32)
    misc[:, 0] = tempq
    misc[:, 1] = tempp
    misc[:, 2] = alpha
    misc[:, 3] = 1.0 - alpha

    shared = {
        "wq": np.ascontiguousarray(q_w.T.astype(np.float16)),
        "wkv": np.ascontiguousarray(kv_w.T.astype(np.float16)),
        "projr": np.ascontiguousarray(proj_w.T.astype(np.float16)),
        "dwc": dwc,
        "miscA": np.ascontiguousarray(misc[0:96]),
        "miscB": np.ascontiguousarray(misc[96:192]),
        "ident": np.eye(128, dtype=np.float16),
        "ones96": np.ones((1, 96), np.float32),
        "dmask": dmask,
        "dwdiag": dwdiag,
    }
    in_maps = []
    for i in range(B):
        im = dict(shared)
        im["x"] = np.ascontiguousarray(x[i].reshape(C, N).astype(np.float16))
        im["y"] = np.ascontiguousarray(y[i].reshape(C, N).astype(np.float16))
        in_maps.append(im)
    return in_maps, full_path


def kernel(**inputs) -> np.ndarray:
    in_maps, full_path = _prep(inputs)
    if full_path not in _CACHE:
        _CACHE[full_path] = build_nc(full_path)
    nc = _CACHE[full_path]
    res = run_bass_kernel_spmd(nc, in_maps, list(range(B)))
    out = np.stack([res.results[i]["out"].reshape(C, H, W) for i in range(B)])
    return out.astype(np.float32)


if __name__ == "__main__":
    import reference
    inputs = reference.setup_inputs()
    expected = np.asarray(reference.reference(**inputs))
    actual = kernel(**{k: np.asarray(v) for k, v in inputs.items()})
    err = np.abs(actual - expected).max() / (np.abs(expected).max() + 1e-30)
    print("Relative error:", err)


# revision 25
# speedup vs baseline: 2.0842x; 1.6704x over previous
"""Trainium2 Bass kernel for nn_CAB (channel-attention block).

8-way batch-parallel (1 sample per NeuronCore). Per core, fused pipeline:
  conv1x1 (PE, fp16) -> depthwise 3x3 (DVE STT chains, fp16 data / fp32 accum)
  -> PE transposes -> gram S=q@k^T accumulated in PSUM over all 16384 pixels
  -> row/col l2 normalization + per-head softmax (exact, fp32)
  -> fold proj_w through the attention matrix (W_effT) -> out = W_eff @ v.

Math identity used: with attn A (block-diag per head), alpha blending and the
final 1x1 proj conv collapse into one matrix:
  out = proj @ (diag(alpha) A1_bd + diag(1-alpha) A2_bd) @ v = W_eff @ v
so branch-2 work is only needed when alpha != 1 (checked at runtime).
"""

import sys

sys.path.insert(0, "/opt/trn_rl_repo")

import numpy as np
from contextlib import ExitStack

import concourse.bass as bass
import concourse.bacc as bacc
import concourse.tile as tile
import concourse.mybir as mybir
from concourse.bass_utils import run_bass_kernel_spmd

F16 = mybir.dt.float16
F32 = mybir.dt.float32
ALU = mybir.AluOpType
AFT = mybir.ActivationFunctionType

B, C, H, W, HEADS = 8, 192, 128, 128, 8
DH = C // HEADS          # 24
N = H * W                # 16384
MB = 16                  # image rows per megablock
NMB = H // MB            # 8
PADW = W + 2             # 130
SLOTS = MB + 2           # 18 row-slots in padded pre-buffers (halo +-1)
MBF = MB * W             # 2048 free elems per megablock

_CACHE = {}


def _dw_cols(w, order="rc"):
    # (ch,1,3,3) -> (ch,9) fp32, tap t=(dy+1)*3+(dx+1)
    return np.ascontiguousarray(w[:, 0].reshape(w.shape[0], 9).astype(np.float32))


def build_nc(full_path: bool, dbg: bool = False):
    nc = bacc.Bacc("TRN2", target_bir_lowering=False, debug=False, num_devices=8)

    x_d = nc.dram_tensor("x", [C, N], F16, kind="ExternalInput")
    y_d = nc.dram_tensor("y", [C, N], F16, kind="ExternalInput")
    wq_d = nc.dram_tensor("wq", [C, C], F16, kind="ExternalInput")       # [cin, cout]
    wkv_d = nc.dram_tensor("wkv", [C, 2 * C], F16, kind="ExternalInput")  # [cin, cout]
    projr_d = nc.dram_tensor("projr", [C, C], F16, kind="ExternalInput")  # [mid, o]
    dwc_d = nc.dram_tensor("dwc", [1024, 10], F32, kind="ExternalInput")
    miscA_d = nc.dram_tensor("miscA", [96, 8], F32, kind="ExternalInput")
    miscB_d = nc.dram_tensor("miscB", [96, 8], F32, kind="ExternalInput")
    ident_d = nc.dram_tensor("ident", [128, 128], F16, kind="ExternalInput")
    ones_d = nc.dram_tensor("ones96", [1, 96], F32, kind="ExternalInput")
    dmask_d = nc.dram_tensor("dmask", [96, 384], F16, kind="ExternalInput")
    dwdiag_d = nc.dram_tensor("dwdiag", [128, 2880], F16, kind="ExternalInput")
    out_d = nc.dram_tensor("out", [C, N], F32, kind="ExternalOutput")
    if dbg:
        dbg_qpre = nc.dram_tensor("dbg_qpre", [128, SLOTS * PADW], F16,
                                  kind="ExternalOutput")
        dbg_qdw = nc.dram_tensor("dbg_qdw", [128, MBF], F16, kind="ExternalOutput")
        dbg_v0 = nc.dram_tensor("dbg_v0", [128, N], F16, kind="ExternalOutput")
        dbg_v1 = nc.dram_tensor("dbg_v1", [64, N], F16, kind="ExternalOutput")
        dbg_S = nc.dram_tensor("dbg_S", [96, 384], F32, kind="ExternalOutput")
        dbg_n = nc.dram_tensor("dbg_n", [128, 3], F32, kind="ExternalOutput")
        dbg_A = nc.dram_tensor("dbg_A", [96, 384], F32, kind="ExternalOutput")
        dbg_We0 = nc.dram_tensor("dbg_We0", [128, 192], F16, kind="ExternalOutput")
        dbg_We1 = nc.dram_tensor("dbg_We1", [64, 192], F16, kind="ExternalOutput")
        dbg_qT = nc.dram_tensor("dbg_qT", [128, 768], F16, kind="ExternalOutput")

    with tile.TileContext(nc) as tc, ExitStack() as ctx:
        const = ctx.enter_context(tc.tile_pool(name="const", bufs=1))
        pers = ctx.enter_context(tc.tile_pool(name="pers", bufs=1))
        xio = ctx.enter_context(tc.tile_pool(name="xio", bufs=(2 if full_path else 3)))
        stg = ctx.enter_context(tc.tile_pool(name="stg", bufs=2))
        convps = ctx.enter_context(tc.tile_pool(name="convps", bufs=2, space="PSUM"))
        # PSUM bank budget: convps(2-3) + trps(2) + gramps(2 or 4) <= 8.
        # Phase-2 psum tiles reuse the S1a/S1b tags (sequential lifetimes).
        trps = ctx.enter_context(tc.tile_pool(name="trps", bufs=1, space="PSUM"))
        gramps = ctx.enter_context(tc.tile_pool(name="gramps", bufs=1, space="PSUM"))
        pb_ = 1 if full_path else 2
        dwsc = ctx.enter_context(tc.tile_pool(name="dwsc", bufs=pb_))
        dwout = ctx.enter_context(tc.tile_pool(name="dwout", bufs=pb_))
        tsb = ctx.enter_context(tc.tile_pool(name="tsb", bufs=(1 if full_path else 2)))
        small = ctx.enter_context(tc.tile_pool(name="small", bufs=1))

        # ---------------- constants into SBUF ----------------
        def cload(name, shape, dt, src_ap):
            t = const.tile(shape, dt, tag=name)
            nc.sync.dma_start(t[:], src_ap)
            return t

        wq0 = cload("wq0", [128, C], F16, wq_d[0:128, :])
        wq1 = cload("wq1", [64, C], F16, wq_d[128:192, :])
        wkv0 = cload("wkv0", [128, 2 * C], F16, wkv_d[0:128, :])
        wkv1 = cload("wkv1", [64, 2 * C], F16, wkv_d[128:192, :])
        projrA = cload("projrA", [96, C], F16, projr_d[0:96, :])
        projrB = cload("projrB", [96, C], F16, projr_d[96:192, :])
        ident = cload("ident", [128, 128], F16, ident_d[:, :])
        ones96 = cload("ones96", [1, 96], F32, ones_d[:, :])
        dmask = cload("dmask", [96, 384], F16, dmask_d[:, :])
        dwdiag = cload("dwdiag", [128, 2880], F16, dwdiag_d[:, :])
        miscA = cload("miscA", [96, 8], F32, miscA_d[:, :])
        miscB = cload("miscB", [96, 8], F32, miscB_d[:, :])
        # dw scalar columns: row blocks of 128 in dwc: 0:q0 1:k0 2:v0 3:q1k1
        # 4:v1 5:pos0 6:pos1
        dwq0 = cload("dwq0", [128, 10], F32, dwc_d[0:128, :])
        dwk0 = cload("dwk0", [128, 10], F32, dwc_d[128:256, :])
        dwv0 = cload("dwv0", [128, 10], F32, dwc_d[256:384, :])
        dwq1k1 = cload("dwq1k1", [128, 10], F32, dwc_d[384:512, :])
        dwv1 = cload("dwv1", [64, 10], F32, dwc_d[512:576, :])
        if full_path:
            dwp0 = cload("dwp0", [128, 10], F32, dwc_d[640:768, :])
            dwp1 = cload("dwp1", [64, 10], F32, dwc_d[768:832, :])

        # ---------------- persistent state ----------------
        vres0 = pers.tile([128, N], F16, tag="vres0")
        vres1 = pers.tile([64, N], F16, tag="vres1")

        def prebuf(name, parts):
            bufs = []
            for i in range(2):
                t = pers.tile([parts, SLOTS * PADW], F16, tag=f"{name}{i}")
                # zero the W-pad columns (cols 0 and 129 of each row slot)
                pr = t[:].rearrange("p (r w) -> p r w", w=PADW)
                nc.gpsimd.memset(pr[:, :, 0:1], 0.0)
                nc.gpsimd.memset(pr[:, :, PADW - 1:PADW], 0.0)
                bufs.append(t)
            return bufs

        pb_q0 = prebuf("pbq0", 128)
        pb_k0 = prebuf("pbk0", 128)
        pb_v0 = prebuf("pbv0", 128)
        pb_q1k1 = prebuf("pbq1k1", 128)
        pb_v1 = prebuf("pbv1", 64)
        if full_path:
            pb_p0 = prebuf("pbp0", 128)
            pb_p1 = prebuf("pbp1", 64)

        nrm_q0 = pers.tile([128, NMB], F32, tag="nrmq0")
        nrm_k0 = pers.tile([128, NMB], F32, tag="nrmk0")
        nrm_q1k1 = pers.tile([128, NMB], F32, tag="nrmq1k1")
        if full_path:
            nrm_p0 = pers.tile([128, NMB], F32, tag="nrmp0")
            nrm_p1 = pers.tile([64, NMB], F32, tag="nrmp1")

        S1a = gramps.tile([96, 192], F32, tag="S1a")
        S1b = gramps.tile([96, 192], F32, tag="S1b")
        if full_path:
            S2a = gramps.tile([96, 192], F32, tag="S2a")
            S2b = gramps.tile([96, 192], F32, tag="S2b")

        # =========== PHASE 1: software-pipelined over megablocks ===========
        def all_pbs():
            l = [(pb_q0, 128), (pb_k0, 128), (pb_v0, 128), (pb_q1k1, 128),
                 (pb_v1, 64)]
            if full_path:
                l += [(pb_p0, 128), (pb_p1, 64)]
            return l

        def emit_conv(m):
            bi = m % 2
            for sti in range(4):
                r0 = MB * m + sti * 4
                n0 = r0 * W
                slot0 = sti * 4 + 1
                ncols = 512

                xa = xio.tile([128, 512], F16, tag="xa")
                xb = xio.tile([64, 512], F16, tag="xb")
                ya = xio.tile([128, 512], F16, tag="ya")
                yb = xio.tile([64, 512], F16, tag="yb")
                nc.sync.dma_start(xa[:], x_d[0:128, n0:n0 + ncols])
                nc.sync.dma_start(xb[:], x_d[128:192, n0:n0 + ncols])
                nc.sync.dma_start(ya[:], y_d[0:128, n0:n0 + ncols])
                nc.sync.dma_start(yb[:], y_d[128:192, n0:n0 + ncols])

                def slot_ap(pb_t, parts, s0):
                    r = pb_t[0:parts, :].rearrange("p (r w) -> p r w", w=PADW)
                    return r[:, s0:s0 + 4, 1:1 + W]

                def conv_piece(rhs_a, rhs_b, w0, w1, mo, msz, dst_ap,
                               via_dma=False):
                    ps = convps.tile([128, 512], F32, tag="cps")
                    o = ps[0:msz, :]
                    nc.tensor.matmul(o, w0[:, mo:mo + msz], rhs_a[:],
                                     start=True, stop=False)
                    nc.tensor.matmul(o, w1[:, mo:mo + msz], rhs_b[:],
                                     start=False, stop=True)
                    if not via_dma:
                        nc.scalar.copy(dst_ap, o.rearrange("p (r w) -> p r w", w=W))
                    else:
                        s = stg.tile([64, 512], F16, tag="kstg")
                        nc.scalar.copy(s[:], o)
                        nc.sync.dma_start(
                            dst_ap, s[:].rearrange("p (r w) -> p r w", w=W))

                conv_piece(xa, xb, wq0, wq1, 0, 128,
                           slot_ap(pb_q0[bi], 128, slot0))
                conv_piece(xa, xb, wq0, wq1, 128, 64,
                           slot_ap(pb_q1k1[bi], 64, slot0))
                conv_piece(ya, yb, wkv0, wkv1, 0, 128,
                           slot_ap(pb_k0[bi], 128, slot0))
                # k1 -> partitions 64:128 of pb_q1k1 via SBUF staging + DMA
                k1_dst = pb_q1k1[bi][64:128, :].rearrange(
                    "p (r w) -> p r w", w=PADW)[:, slot0:slot0 + 4, 1:1 + W]
                conv_piece(ya, yb, wkv0, wkv1, 128, 64, k1_dst, via_dma=True)
                conv_piece(ya, yb, wkv0, wkv1, 192, 128,
                           slot_ap(pb_v0[bi], 128, slot0))
                conv_piece(ya, yb, wkv0, wkv1, 320, 64,
                           slot_ap(pb_v1[bi], 64, slot0))
                if full_path:
                    nc.scalar.copy(slot_ap(pb_p0[bi], 128, slot0),
                                   xa[:].rearrange("p (r w) -> p r w", w=W))
                    nc.scalar.copy(slot_ap(pb_p1[bi], 64, slot0),
                                   xb[0:64, :].rearrange("p (r w) -> p r w", w=W))

        def emit_halo(m):
            # after conv(m): fill slot 0 of buf m (last row of mb m-1) and
            # slot 17 of buf m-1 (first row of mb m)
            bi, pi = m % 2, (m - 1) % 2
            for pb, parts in all_pbs():
                cur = pb[bi][0:parts, :].rearrange("p (r w) -> p r w", w=PADW)
                if m == 0:
                    nc.gpsimd.memset(cur[:, 0:1, :], 0.0)
                else:
                    prev = pb[pi][0:parts, :].rearrange("p (r w) -> p r w", w=PADW)
                    nc.sync.dma_start(cur[:, 0:1, :],
                                      prev[:, SLOTS - 2:SLOTS - 1, :])
                    nc.sync.dma_start(prev[:, SLOTS - 1:SLOTS, :],
                                      cur[:, 1:2, :])
                if m == NMB - 1:
                    nc.gpsimd.memset(cur[:, SLOTS - 1:SLOTS, :], 0.0)

        def dw_win(pb_t, parts, t):
            pr = pb_t[0:parts, :].rearrange("p (r w) -> p r w", w=PADW)
            dy, dx = t // 3 - 1, t % 3 - 1
            return pr[:, 1 + dy:1 + dy + MB, 1 + dx:1 + dx + W]

        def dw_stt(pb_t, parts, wcol, dst_ap, bias_col=None):
            # 9 fused MAC ops on DVE (1x rate), fp32 accumulate
            acc_t = dwsc.tile([128, MBF], F32, tag="acc")
            acc = acc_t[0:parts, :].rearrange("p (r w) -> p r w", w=W)
            if bias_col is None:
                nc.vector.tensor_scalar(acc, dw_win(pb_t, parts, 0),
                                        wcol[:, 0:1], None, ALU.mult)
            else:
                nc.vector.tensor_scalar(acc, dw_win(pb_t, parts, 0),
                                        wcol[:, 0:1], bias_col,
                                        ALU.mult, ALU.add)
            for t in range(1, 8):
                nc.vector.scalar_tensor_tensor(
                    acc, dw_win(pb_t, parts, t), wcol[:, t:t + 1], acc,
                    ALU.mult, ALU.add)
            nc.vector.scalar_tensor_tensor(
                dst_ap, dw_win(pb_t, parts, 8), wcol[:, 8:9], acc,
                ALU.mult, ALU.add)

        def dw_tree(pb_t, parts, wcol, dst_ap, r0=0, nrows=MB):
            # DVE: 9 tensor_scalar (4x fp16) + 8 tensor_tensor adds (2x)
            fd = nrows * W

            def win(t):
                pr = pb_t[0:parts, :].rearrange("p (r w) -> p r w", w=PADW)
                dy, dx = t // 3 - 1, t % 3 - 1
                return pr[:, 1 + r0 + dy:1 + r0 + dy + nrows,
                          1 + dx:1 + dx + W]

            sA_t = dwsc.tile([128, MBF], F16, tag="tA")
            sB_t = dwsc.tile([128, MBF], F16, tag="tB")
            sA = sA_t[0:parts, 0:fd].rearrange("p (r w) -> p r w", w=W)
            sB = sB_t[0:parts, 0:fd].rearrange("p (r w) -> p r w", w=W)
            nc.vector.tensor_scalar(sA, win(0), wcol[:, 0:1], None, ALU.mult)
            for t in range(1, 8):
                nc.vector.tensor_scalar(sB, win(t), wcol[:, t:t + 1],
                                        None, ALU.mult)
                nc.vector.tensor_tensor(sA, sA, sB, ALU.add)
            nc.vector.tensor_scalar(sB, win(8), wcol[:, 8:9], None, ALU.mult)
            nc.vector.tensor_tensor(dst_ap, sA, sB, ALU.add)

        def dw_pe(pb_t, parts, diag_off, dst_ap_fn, groups=(0, 1, 2, 3)):
            # 9 accumulating diag matmuls per 4-row window on PE, ACT evicts
            pr = pb_t[0:parts, :].rearrange("p (r w) -> p r w", w=PADW)
            for gi in groups:
                ps = convps.tile([128, 512], F32, tag="dwps")
                for t in range(9):
                    dy, dx = t // 3 - 1, t % 3 - 1
                    s0 = 1 + 4 * gi + dy
                    rhs = pr[:, s0:s0 + 4, 1 + dx:1 + dx + W]
                    nc.tensor.matmul(
                        ps[0:parts, :],
                        dwdiag[0:parts,
                               diag_off + t * parts:diag_off + (t + 1) * parts],
                        rhs, start=(t == 0), stop=(t == 8))
                nc.scalar.copy(dst_ap_fn(gi), ps[0:parts, :])

        def emit_process(m):
            bi = m % 2
            if dbg and m == 0:
                nc.sync.dma_start(dbg_qpre[:, :], pb_q0[bi][:])
            qdw = dwout.tile([128, MBF], F16, tag="qdw")
            kdw = dwout.tile([128, MBF], F16, tag="kdw")
            q1k1dw = dwout.tile([128, MBF], F16, tag="q1k1dw")
            r128 = lambda ap: ap.rearrange("p (r w) -> p r w", w=W)
            dw_tree(pb_q0[bi], 128, dwq0, r128(qdw[:]))
            dw_tree(pb_k0[bi], 128, dwk0, r128(kdw[:]))
            dw_pe(pb_q1k1[bi], 128, 1152,
                  lambda gi: q1k1dw[:, gi * 512:(gi + 1) * 512])
            dw_pe(pb_v0[bi], 128, 0,
                  lambda gi: vres0[:, m * MBF + gi * 512:m * MBF + (gi + 1) * 512])
            dw_pe(pb_v1[bi], 64, 2304,
                  lambda gi: vres1[:, m * MBF + gi * 512:m * MBF + (gi + 1) * 512])
            if full_path:
                pdw = dwout.tile([128, MBF], F16, tag="pdw")
                p1dw = dwout.tile([64, MBF], F16, tag="p1dw")
                dw_stt(pb_p0[bi], 128, dwp0, r128(pdw[:]),
                       bias_col=dwp0[:, 9:10])
                dw_stt(pb_p1[bi], 64, dwp1, r128(p1dw[:]),
                       bias_col=dwp1[:, 9:10])

            if dbg and m == 0:
                nc.sync.dma_start(dbg_qdw[:, :], qdw[:])

            # norms (sum of squares per channel) on ACT
            def sq_accum(src_ap, parts, dst_col):
                scr = dwsc.tile([128, MBF], F16, tag="sqscr")
                nc.scalar.activation(scr[0:parts, :], src_ap, AFT.Square,
                                     accum_out=dst_col)
            sq_accum(qdw[:], 128, nrm_q0[:, m:m + 1])
            sq_accum(kdw[:], 128, nrm_k0[:, m:m + 1])
            sq_accum(q1k1dw[:], 128, nrm_q1k1[:, m:m + 1])
            if full_path:
                sq_accum(pdw[:], 128, nrm_p0[:, m:m + 1])
                sq_accum(p1dw[:], 64, nrm_p1[:, m:m + 1])

            # transposes (PE) + gram accumulation
            for g in range(4):
                qT_ps = trps.tile([128, 768], F16, tag="qTps")
                kT_ps = trps.tile([128, 768], F16, tag="kTps")
                for r4 in range(4):
                    r = g * 4 + r4
                    sl = slice(r * W, (r + 1) * W)
                    co = r4 * 192
                    nc.tensor.transpose(qT_ps[:, co:co + 128], qdw[:, sl],
                                        ident[:, :])
                    nc.tensor.transpose(qT_ps[:, co + 128:co + 192],
                                        q1k1dw[0:64, sl], ident[0:64, 0:64])
                    nc.tensor.transpose(kT_ps[:, co:co + 128], kdw[:, sl],
                                        ident[:, :])
                    nc.tensor.transpose(kT_ps[:, co + 128:co + 192],
                                        q1k1dw[64:128, sl],
                                        ident[64:128, 64:128])
                qT = tsb.tile([128, 768], F16, tag="qT")
                kT = tsb.tile([128, 768], F16, tag="kT")
                nc.vector.tensor_copy(qT[:], qT_ps[:])
                nc.vector.tensor_copy(kT[:], kT_ps[:])
                if full_path:
                    pT_ps = trps.tile([128, 768], F16, tag="qTps")
                    for r4 in range(4):
                        r = g * 4 + r4
                        sl = slice(r * W, (r + 1) * W)
                        co = r4 * 192
                        nc.tensor.transpose(pT_ps[:, co:co + 128], pdw[:, sl],
                                            ident[:, :])
                        nc.tensor.transpose(pT_ps[:, co + 128:co + 192],
                                            p1dw[:, sl], ident[0:64, 0:64])
                    pT = tsb.tile([128, 768], F16, tag="pT")
                    nc.vector.tensor_copy(pT[:], pT_ps[:])
                if dbg and m == 0 and g == 0:
                    nc.sync.dma_start(dbg_qT[:, :], qT[:])
                for r4 in range(4):
                    row = m * MB + g * 4 + r4
                    st = row == 0
                    sp = row == H - 1
                    co = r4 * 192
                    nc.tensor.matmul(S1a[:], qT[:, co:co + 96],
                                     kT[:, co:co + 192], start=st, stop=sp)
                    nc.tensor.matmul(S1b[:], qT[:, co + 96:co + 192],
                                     kT[:, co:co + 192], start=st, stop=sp)
                    if full_path:
                        nc.tensor.matmul(S2a[:], pT[:, co:co + 96],
                                         kT[:, co:co + 192], start=st, stop=sp)
                        nc.tensor.matmul(S2b[:], pT[:, co + 96:co + 192],
                                         kT[:, co:co + 192], start=st, stop=sp)

        for m in range(NMB):
            emit_conv(m)
            emit_halo(m)
            if m >= 1:
                emit_process(m - 1)
        emit_process(NMB - 1)

        # =========== PHASE 2: softmax + W_eff fold (small) ===========
        # Evacuate gram accumulators first so their PSUM tags can be reused.
        Ssb1 = small.tile([96, 384], F32, tag="Ssb1")
        nc.scalar.copy(Ssb1[:, 0:192], S1a[:])
        nc.scalar.copy(Ssb1[:, 192:384], S1b[:])
        if full_path:
            Ssb2 = small.tile([96, 384], F32, tag="Ssb2")
            nc.scalar.copy(Ssb2[:, 0:192], S2a[:])
            nc.scalar.copy(Ssb2[:, 192:384], S2b[:])
        # reduce per-mb sumsq columns -> n^2 per channel
        nq0 = small.tile([128, 1], F32, tag="nq0")
        nk0 = small.tile([128, 1], F32, tag="nk0")
        nq1k1 = small.tile([128, 1], F32, tag="nq1k1")
        nc.vector.tensor_reduce(nq0[:], nrm_q0[:], mybir.AxisListType.X, ALU.add)
        nc.vector.tensor_reduce(nk0[:], nrm_k0[:], mybir.AxisListType.X, ALU.add)
        nc.vector.tensor_reduce(nq1k1[:], nrm_q1k1[:], mybir.AxisListType.X, ALU.add)
        if full_path:
            np0 = small.tile([128, 1], F32, tag="np0")
            np1 = small.tile([64, 1], F32, tag="np1")
            nc.vector.tensor_reduce(np0[:], nrm_p0[:], mybir.AxisListType.X, ALU.add)
            nc.vector.tensor_reduce(np1[:], nrm_p1[:], mybir.AxisListType.X, ALU.add)

        _rs_ctr = [0]

        def rsqrt_col(dst, src_ap, parts):
            # dst = 1 / max(sqrt(src), 1e-12)
            _rs_ctr[0] += 1
            t = small.tile([128, 1], F32, tag=f"rs{_rs_ctr[0]}")
            nc.scalar.sqrt(t[0:parts, :], src_ap)
            nc.vector.tensor_scalar_max(t[0:parts, :], t[0:parts, :], 1e-12)
            nc.vector.reciprocal(dst, t[0:parts, :])
            return dst

        if dbg:
            nc.sync.dma_start(dbg_v0[:, :], vres0[:])
            nc.sync.dma_start(dbg_v1[:, :], vres1[:])
            nc.sync.dma_start(dbg_S[:, :], Ssb1[:])
            nc.sync.dma_start(dbg_n[:, 0:1], nq0[:])
            nc.sync.dma_start(dbg_n[:, 1:2], nk0[:])
            nc.sync.dma_start(dbg_n[:, 2:3], nq1k1[:])
        # q-row scales, head-aligned halves [96,1]
        rqa = small.tile([96, 1], F32, tag="rqa")
        rqb = small.tile([96, 1], F32, tag="rqb")
        nqb = small.tile([96, 1], F32, tag="nqb")
        nc.sync.dma_start(nqb[0:32, :], nq0[96:128, :])
        nc.sync.dma_start(nqb[32:96, :], nq1k1[0:64, :])
        rsqrt_col(rqa[:], nq0[0:96, :], 96)
        rsqrt_col(rqb[:], nqb[:], 96)
        # fold temp1 (per q-channel) into the row scale
        nc.vector.tensor_tensor(rqa[:], rqa[:], miscA[:, 0:1], ALU.mult)
        nc.vector.tensor_tensor(rqb[:], rqb[:], miscB[:, 0:1], ALU.mult)

        # k-col scales as a broadcast tile [96,192]
        nk1 = small.tile([64, 1], F32, tag="nk1")
        nc.sync.dma_start(nk1[:], nq1k1[64:128, :])
        # cast the norm columns to f16 so the PE transpose dtype matches ident
        nk0h = small.tile([128, 1], F16, tag="nk0h")
        nk1h = small.tile([64, 1], F16, tag="nk1h")
        nc.scalar.copy(nk0h[:], nk0[:])
        nc.scalar.copy(nk1h[:], nk1[:])
        rk_ps = gramps.tile([1, 192], F16, tag="S1a")
        nc.tensor.transpose(rk_ps[:, 0:128], nk0h[:], ident[:, :])
        nc.tensor.transpose(rk_ps[:, 128:192], nk1h[:], ident[0:64, 0:64])
        rk_row = small.tile([1, 192], F32, tag="rkrow")
        nc.scalar.sqrt(rk_row[:], rk_ps[:])
        nc.vector.tensor_scalar_max(rk_row[:], rk_row[:], 1e-12)
        nc.vector.reciprocal(rk_row[:], rk_row[:])
        rkb_ps = gramps.tile([96, 192], F32, tag="S1b")
        nc.tensor.matmul(rkb_ps[:], ones96[:], rk_row[:], start=True, stop=True)
        rkb = small.tile([96, 192], F32, tag="rkb")
        nc.scalar.copy(rkb[:], rkb_ps[:])

        def softmax_block(Ssb, rqa_c, rqb_c, tag):
            # Ssb [96,384]: cols 0:192 = q-rows 0:96, 192:384 = q-rows 96:192
            for half, rq_c in ((0, rqa_c), (192, rqb_c)):
                h = Ssb[:, half:half + 192]
                nc.vector.tensor_tensor(h, h, rkb[:], ALU.mult)
                nc.scalar.mul(h, h, rq_c)
            ex = small.tile([96, 384], F32, tag=f"ex_{tag}")
            nc.scalar.activation(ex[:], Ssb[:], AFT.Exp)
            sums = small.tile([96, 16], F32, tag=f"sums_{tag}")
            nc.vector.tensor_reduce(
                sums[:], ex[:].rearrange("p (h j) -> p h j", j=DH),
                mybir.AxisListType.X, ALU.add)
            nc.vector.reciprocal(sums[:], sums[:])
            A = small.tile([96, 384], F32, tag=f"A_{tag}")
            for blk in range(16):
                nc.vector.tensor_scalar_mul(
                    A[:, blk * DH:(blk + 1) * DH], ex[:, blk * DH:(blk + 1) * DH],
                    sums[:, blk:blk + 1])
            return A

        A1 = softmax_block(Ssb1, rqa[:], rqb[:], "a1")

        if dbg:
            nc.sync.dma_start(dbg_A[:, :], A1[:])
        # M_bd [mid, i] block-diagonal, fp16, two partition halves.
        # Build by masking the full softmax tiles (no partition-24 slicing).
        # M1a[mid 0:96, i] = A1a * maskA ; M1b[mid 96:192, i] = A1b * maskB
        M1a = small.tile([96, 192], F16, tag="M1a")
        M1b = small.tile([96, 192], F16, tag="M1b")
        nc.vector.tensor_tensor(M1a[:], A1[:, 0:192], dmask[:, 0:192], ALU.mult)
        nc.vector.tensor_tensor(M1b[:], A1[:, 192:384], dmask[:, 192:384], ALU.mult)

        if full_path:
            # pos-branch scales
            rpa = small.tile([96, 1], F32, tag="rpa")
            rpb = small.tile([96, 1], F32, tag="rpb")
            npb = small.tile([96, 1], F32, tag="npb")
            nc.sync.dma_start(npb[0:32, :], np0[96:128, :])
            nc.sync.dma_start(npb[32:96, :], np1[0:64, :])
            rsqrt_col(rpa[:], np0[0:96, :], 96)
            rsqrt_col(rpb[:], npb[:], 96)
            nc.vector.tensor_tensor(rpa[:], rpa[:], miscA[:, 1:2], ALU.mult)
            nc.vector.tensor_tensor(rpb[:], rpb[:], miscB[:, 1:2], ALU.mult)
            A2 = softmax_block(Ssb2, rpa[:], rpb[:], "a2")
            M2a = small.tile([96, 192], F16, tag="M2a")
            M2b = small.tile([96, 192], F16, tag="M2b")
            nc.vector.tensor_tensor(M2a[:], A2[:, 0:192], dmask[:, 0:192], ALU.mult)
            nc.vector.tensor_tensor(M2b[:], A2[:, 192:384], dmask[:, 192:384], ALU.mult)
            # M = diag(alpha) M1 + diag(1-alpha) M2   (per mid-channel)
            t1 = small.tile([96, 192], F32, tag="mca")
            for Ma, Mb_, mi in ((M1a, M2a, miscA), (M1b, M2b, miscB)):
                nc.vector.tensor_scalar_mul(t1[:], Ma[:], mi[:, 2:3])
                nc.vector.tensor_scalar_mul(Mb_[:], Mb_[:], mi[:, 3:4])
                nc.vector.tensor_tensor(Ma[:], t1[:], Mb_[:], ALU.add)

        # W_effT[i, o] = sum_mid M_bd[mid, i] * projr[mid, o]
        WeT_ps0 = gramps.tile([128, 192], F32, tag="S1a")
        WeT_ps1 = gramps.tile([64, 192], F32, tag="S1b")
        for isl, msz, ps in ((0, 128, WeT_ps0), (128, 64, WeT_ps1)):
            nc.tensor.matmul(ps[:], M1a[:, isl:isl + msz], projrA[:],
                             start=True, stop=False)
            nc.tensor.matmul(ps[:], M1b[:, isl:isl + msz], projrB[:],
                             start=False, stop=True)
        WeT0 = small.tile([128, 192], F16, tag="WeT0")
        WeT1 = small.tile([64, 192], F16, tag="WeT1")
        nc.scalar.copy(WeT0[:], WeT_ps0[:])
        nc.scalar.copy(WeT1[:], WeT_ps1[:])

        if dbg:
            nc.sync.dma_start(dbg_We0[:, :], WeT0[:])
            nc.sync.dma_start(dbg_We1[:, :], WeT1[:])
        # =========== PHASE 3: out = W_eff @ v ===========
        for t in range(N // 512):
            sl = slice(t * 512, (t + 1) * 512)
            big = convps.tile([128, 512], F32, tag="cps")
            sm = convps.tile([64, 512], F32, tag="cps")
            nc.tensor.matmul(big[:], WeT0[:, 0:128], vres0[:, sl],
                             start=True, stop=False)
            nc.tensor.matmul(big[:], WeT1[:, 0:128], vres1[:, sl],
                             start=False, stop=True)
            nc.tensor.matmul(sm[:], WeT0[:, 128:192], vres0[:, sl],
                             start=True, stop=False)
            nc.tensor.matmul(sm[:], WeT1[:, 128:192], vres1[:, sl],
                             start=False, stop=True)
            ob = stg.tile([128, 512], F32, tag="ob")
            os_ = stg.tile([64, 512], F32, tag="os")
            nc.scalar.copy(ob[:], big[:])
            nc.vector.tensor_copy(os_[:], sm[:])
            nc.sync.dma_start(out_d[0:128, sl], ob[:])
            nc.sync.dma_start(out_d[128:192, sl], os_[:])

    nc.compile()
    return nc


def _prep(inputs):
    x = np.asarray(inputs["x"], np.float32)
    y = np.asarray(inputs["y"], np.float32)
    q_w = np.asarray(inputs["q_w"], np.float32)[:, :, 0, 0]      # [out,in]
    kv_w = np.asarray(inputs["kv_w"], np.float32)[:, :, 0, 0]    # [2C,in]
    proj_w = np.asarray(inputs["proj_w"], np.float32)[:, :, 0, 0]
    q_dw = _dw_cols(np.asarray(inputs["q_dw_w"], np.float32))
    kv_dw = _dw_cols(np.asarray(inputs["kv_dw_w"], np.float32))
    pos_dw = _dw_cols(np.asarray(inputs["pos_conv_w"], np.float32))
    temp1 = np.asarray(inputs["temp1"], np.float32).reshape(HEADS)
    temp2 = np.asarray(inputs["temp2"], np.float32).reshape(HEADS)
    alpha = np.asarray(inputs["alpha"], np.float32).reshape(C)
    pos_embed = np.asarray(inputs["pos_embed"], np.float32).reshape(DH)

    full_path = not (np.all(alpha == 1.0))

    k_dw, v_dw = kv_dw[0:C], kv_dw[C:2 * C]
    dwc = np.zeros((1024, 10), np.float32)
    dwc[0:128, 0:9] = q_dw[0:128]
    dwc[128:256, 0:9] = k_dw[0:128]
    dwc[256:384, 0:9] = v_dw[0:128]
    dwc[384:448, 0:9] = q_dw[128:192]
    dwc[448:512, 0:9] = k_dw[128:192]
    dwc[512:576, 0:9] = v_dw[128:192]
    pe_col = np.tile(pos_embed, HEADS)  # per-channel pos_embed
    dwc[640:768, 0:9] = pos_dw[0:128]
    dwc[640:768, 9] = pe_col[0:128]
    dwc[768:832, 0:9] = pos_dw[128:192]
    dwc[768:832, 9] = pe_col[128:192]

    dmask = np.zeros((96, 384), np.float16)
    for h in range(4):
        dmask[h * DH:(h + 1) * DH, h * DH:(h + 1) * DH] = 1.0
    for h in range(4, 8):
        dmask[(h - 4) * DH:(h - 3) * DH, 192 + h * DH:192 + (h + 1) * DH] = 1.0
    dwdiag = np.zeros((128, 2880), np.float16)
    q1k1_w = np.concatenate([q_dw[128:192], k_dw[128:192]], 0)  # [128, 9]
    for t in range(9):
        np.fill_diagonal(dwdiag[:, t * 128:(t + 1) * 128], v_dw[0:128, t])
        np.fill_diagonal(dwdiag[:, 1152 + t * 128:1152 + (t + 1) * 128],
                         q1k1_w[:, t])
        np.fill_diagonal(dwdiag[0:64, 2304 + t * 64:2304 + (t + 1) * 64],
                         v_dw[128:192, t])
    tempq = np.repeat(temp1, DH)
    tempp = np.repeat(temp2, DH)
    misc = np.zeros((C, 8), np.float32)
    misc[:, 0] = tempq
    misc[:, 1] = tempp
    misc[:, 2] = alpha
    misc[:, 3] = 1.0 - alpha

    shared = {
        "wq": np.ascontiguousarray(q_w.T.astype(np.float16)),
        "wkv": np.ascontiguousarray(kv_w.T.astype(np.float16)),
        "projr": np.ascontiguousarray(proj_w.T.astype(np.float16)),
        "dwc": dwc,
        "miscA": np.ascontiguousarray(misc[0:96]),
        "miscB": np.ascontiguousarray(misc[96:192]),
        "ident": np.eye(128, dtype=np.float16),
        "ones96": np.ones((1, 96), np.float32),
        "dmask": dmask,
        "dwdiag": dwdiag,
    }
    in_maps = []
    for i in range(B):
        im = dict(shared)
        im["x"] = np.ascontiguousarray(x[i].reshape(C, N).astype(np.float16))
        im["y"] = np.ascontiguousarray(y[i].reshape(C, N).astype(np.float16))
        in_maps.append(im)
    return in_maps, full_path


def kernel(**inputs) -> np.ndarray:
    in_maps, full_path = _prep(inputs)
    if full_path not in _CACHE:
        _CACHE[full_path] = build_nc(full_path)
    nc = _CACHE[full_path]
    res = run_bass_kernel_spmd(nc, in_maps, list(range(B)))
    out = np.stack([res.results[i]["out"].reshape(C, H, W) for i in range(B)])
    return out.astype(np.float32)


if __name__ == "__main__":
    import reference
    inputs = reference.setup_inputs()
    expected = np.asarray(reference.reference(**inputs))
    actual = kernel(**{k: np.asarray(v) for k, v in inputs.items()})
    err = np.abs(actual - expected).max() / (np.abs(expected).max() + 1e-30)
    print("Relative error:", err)
